# revision 13
# baseline (speedup 1.0000x reference)
"""Distributed 3-layer GAT + FC kernel for Trainium2 (8 NeuronCores).

Strategy (graph/data parallel, per the sharding hint):
  - Nodes are assigned to the 8 cores by in-degree rank interleaving
    (rank r -> core r%8, local slot r//8), so every core gets an almost
    identical degree profile and the SPMD program is shared.
  - Each core owns its nodes' incoming edges, laid out degree-bucketed:
    dst node = partition, incoming-edge slot = free-dim column.  One
    indirect DMA (128 rows, one per partition) gathers the source-node
    table rows for one edge-slot column.  Tiles (128 dst nodes) are
    grouped into chunks with a uniform, even slot count Kc so the edge
    math runs as a handful of wide DVE/ACT ops per chunk instead of
    per-head per-tile ops.
  - Per layer, each core projects its own nodes ([h | el | er] in one
    fused matmul; the el/er attention reductions are folded into the
    projection matrix host-side), then an AllGather replicates the
    bf16 [h | el] node table to every core (the halo exchange).
  - Edge softmax: the segment max is skipped (|e| <= ~10 on this data,
    exp cannot overflow) and the alpha normalization is applied after
    the weighted tree-reduction:
        out[n] = (sum_e exp(e_e) h[src_e]) / (sum_e exp(e_e)).
    Padding edge slots point at a pad-node table row whose el is set
    to -1e30, so exp() contributes exactly 0.
  - Table/gather/message datapath is bf16 (relative-error budget
    2e-2); the softmax path and final outputs are fp32.
  - Small weight tensors are replicated; the final FC stays node-local.
"""

import numpy as np

N_NODES = 50000
N_EDGES = 1000000
NC = 8
NPC = N_NODES // NC          # 6250 owned nodes per core
NT = 49                      # node tiles per core (128 nodes each)
NPCP = NT * 128              # 6272 padded nodes per core
NTOT = NPCP * NC             # 50176 table rows
PADROW = 6250                # core 0's first pad slot (el = -1e30)
NEG_SLOPE = 0.2

COLS_MAX = 128               # max G*Kc gathered per chunk
G_MAX = 8                    # max tiles per chunk

# (Fin, H, D) per GAT layer
LAYERS = [(25, 4, 10), (40, 4, 25), (100, 1, 50)]

_cache = {}


def _patch_tile_drain():
    """walrus in this toolchain rejects instructions carrying more than one
    semaphore wait; split the TileContext tail drain's waits onto
    single-wait NOPs."""
    import concourse.tile as tile_mod
    import concourse.mybir as mybir
    from concourse.vector_clock import ScopedClock

    if getattr(tile_mod.TileContext, "_drain_patched", False):
        return

    def _patched(self, tick_clock, wait_clock):
        nc = self.nc
        drain_inst = nc.sync.drain()
        wait_clock.add_sem_waits(
            drain_inst.ins, ScopedClock({None: tick_clock.global_clock})
        )
        si = drain_inst.ins.sync_info
        waits = list(si.on_wait or []) if si is not None else []
        if len(waits) > 1:
            si.on_wait.clear()
            bb = nc.cur_bb.bb
            assert bb.instructions[-1] is drain_inst.ins
            bb.instructions.pop()
            for w in waits:
                nop = nc.sync.nop(nofuse=True, hint="drain_wait_split")
                if nop.ins.sync_info is None:
                    nop.ins.sync_info = mybir.SyncInfo(on_wait=[w], on_update=[])
                else:
                    nop.ins.sync_info.on_wait.append(w)
            bb.add_instruction(drain_inst.ins)
        nc.all_engine_barrier()
        assert self.sems is not None
        popped = nc._tile_sem_poison_stack.pop()
        assert popped is self._sem_poison
        nc.clear_and_free_semaphores(list(self.sems.allocated().values()))
        nc.all_engine_barrier()

    tile_mod.TileContext._drain_and_barrier = _patched
    tile_mod.TileContext._drain_patched = True


def _preprocess(src, dst):
    """Node->core assignment, chunked degree layout, gather indices."""
    deg = np.bincount(dst, minlength=N_NODES)
    rank = np.argsort(-deg, kind="stable")
    node_core = np.empty(N_NODES, np.int64)
    node_loc = np.empty(N_NODES, np.int64)
    node_core[rank] = np.arange(N_NODES) % NC
    node_loc[rank] = np.arange(N_NODES) // NC
    glob = node_core * NPCP + node_loc

    loc_deg = np.zeros((NC, NPCP), np.int64)
    loc_deg[node_core, node_loc] = deg
    K_t = [int(loc_deg[:, t * 128:(t + 1) * 128].max()) for t in range(NT)]

    # group tiles into chunks with a shared, even Kc (K_t is non-increasing)
    chunks = []                      # (t0, G, Kc, col0)
    t = 0
    col = 0
    while t < NT:
        Kc = max(2, K_t[t] + (K_t[t] & 1))
        G = 1
        while (t + G < NT and G < G_MAX and (G + 1) * Kc <= COLS_MAX):
            G += 1
        chunks.append((t, G, Kc, col))
        col += G * Kc
        t += G
    C_tot = col

    idx_arr = np.full((NC, 128, C_tot), PADROW, np.int32)
    key = node_core[dst] * NPCP + node_loc[dst]
    eorder = np.argsort(key, kind="stable")
    ksorted = key[eorder]
    starts = np.searchsorted(ksorted, np.arange(NC * NPCP))
    ends = np.searchsorted(ksorted, np.arange(NC * NPCP) + 1)
    gsorted = glob[src[eorder]]
    for c in range(NC):
        base = c * NPCP
        for (t0, G, Kc, col0) in chunks:
            for g in range(G):
                tt = t0 + g
                cb = col0 + g * Kc
                for u in range(128):
                    loc = base + tt * 128 + u
                    s0, s1 = starts[loc], ends[loc]
                    if s1 > s0:
                        idx_arr[c, u, cb:cb + (s1 - s0)] = gsorted[s0:s1]
    return node_core, node_loc, tuple(chunks), C_tot, idx_arr


def _proj_matrix(W, al, ar):
    """P = [W; L^T W; R^T W] so that x @ P.T = [h | el | er] per node."""
    H, D = al.shape
    HD = H * D
    L = np.zeros((HD, H), np.float32)
    R = np.zeros((HD, H), np.float32)
    for h in range(H):
        L[h * D:(h + 1) * D, h] = al[h]
        R[h * D:(h + 1) * D, h] = ar[h]
    return np.vstack([W, L.T @ W, R.T @ W]).astype(np.float32)


def _build(chunks, C_tot):
    import concourse.bass as bass
    import concourse.bacc as bacc
    import concourse.mybir as mybir
    from concourse.tile import TileContext
    from concourse.masks import make_identity

    _patch_tile_drain()

    nc = bacc.Bacc("TRN2", target_bir_lowering=False, debug=False, num_devices=NC)
    f32 = mybir.dt.float32
    bf16 = mybir.dt.bfloat16

    # ---- external inputs ----
    xin = nc.dram_tensor("xin", [25, NPCP], bf16, kind="ExternalInput")
    idx = nc.dram_tensor("idx", [128, C_tot], mybir.dt.int32, kind="ExternalInput")
    pTs = []
    bts = []
    for l, (Fin, H, D) in enumerate(LAYERS):
        pTs.append(nc.dram_tensor(f"pT{l}", [Fin, H * D + 2 * H], bf16,
                                  kind="ExternalInput"))
        bts.append(nc.dram_tensor(f"bias{l}", [128, H * D], f32,
                                  kind="ExternalInput"))
    fcT = nc.dram_tensor("fcT", [50, 93], bf16, kind="ExternalInput")
    fcb = nc.dram_tensor("fcb", [128, 93], f32, kind="ExternalInput")
    padm = nc.dram_tensor("padm", [128, 4], f32, kind="ExternalInput")
    out93 = nc.dram_tensor("out93", [NPCP, 93], f32, kind="ExternalOutput")

    # ---- internal DRAM: halo pieces and gathered tables (bf16) ----
    pieces = []
    tables = []
    DRs = []
    for l, (Fin, H, D) in enumerate(LAYERS):
        DR = H * D + H
        DR += DR & 1                     # even row width (4B-aligned rows)
        DRs.append(DR)
        pieces.append(nc.dram_tensor(f"piece{l}", [NPCP, DR], bf16,
                                     kind="Internal"))
        tables.append(nc.dram_tensor(f"table{l}", [NTOT, DR], bf16,
                                     kind="Internal", addr_space="Shared"))

    with TileContext(nc) as tc:
        with (
            tc.tile_pool(name="const", bufs=1) as cpool,
            tc.tile_pool(name="pc", bufs=3) as pcp,
            tc.tile_pool(name="gt", bufs=2) as gtp,
            tc.tile_pool(name="mm", bufs=2) as mmp,
            tc.tile_pool(name="wk", bufs=3) as wkp,
            tc.tile_pool(name="psa", bufs=4, space="PSUM") as psa,
            tc.tile_pool(name="psb", bufs=4, space="PSUM") as psb,
        ):
            ident = cpool.tile([128, 128], f32, tag="ident")
            make_identity(nc, ident[:])
            it = cpool.tile([128, C_tot], mybir.dt.int32, tag="idx")
            nc.sync.dma_start(it[:], idx[:])
            pt_t = []
            b_t = []
            for l, (Fin, H, D) in enumerate(LAYERS):
                HD = H * D
                p = cpool.tile([Fin, HD + 2 * H], bf16, tag=f"pt{l}")
                nc.sync.dma_start(p[:], pTs[l][:])
                pt_t.append(p)
                b = cpool.tile([128, HD], f32, tag=f"b{l}")
                nc.sync.dma_start(b[:], bts[l][:])
                b_t.append(b)
            fct = cpool.tile([50, 93], bf16, tag="fct")
            nc.sync.dma_start(fct[:], fcT[:])
            fcbt = cpool.tile([128, 93], f32, tag="fcbt")
            nc.sync.dma_start(fcbt[:], fcb[:])
            padt = cpool.tile([128, 4], f32, tag="padt")
            nc.sync.dma_start(padt[:], padm[:])
            ers = [cpool.tile([128, NT, LAYERS[l][1]], bf16, tag=f"er{l}",
                              name=f"er{l}") for l in range(3)]
            xt0 = cpool.tile([25, NPCP], bf16, tag="xt0")
            nc.sync.dma_start(xt0[:], xin[:])
            xts = [xt0,
                   cpool.tile([40, NPCP], bf16, tag="xt1", name="xt1"),
                   cpool.tile([100, NPCP], bf16, tag="xt2", name="xt2"),
                   cpool.tile([50, NPCP], bf16, tag="xt3", name="xt3")]

            for l, (Fin, H, D) in enumerate(LAYERS):
                HD = H * D
                DR = DRs[l]
                PR = HD + 2 * H
                xt = xts[l]
                xt_next = xts[l + 1]
                table = tables[l]
                piece = pieces[l]
                er_sb = ers[l]

                # ---- projection of own nodes -> piece rows [h | el] ----
                for s in range(NT):
                    cp = psa.tile([128, PR], f32, tag="ps_a", space="PSUM")
                    nc.tensor.matmul(cp[:], lhsT=xt[:, s * 128:(s + 1) * 128],
                                     rhs=pt_t[l][:], start=True, stop=True)
                    pe = pcp.tile([128, DR], bf16, tag="pe")
                    nc.scalar.copy(pe[:, 0:HD], cp[:, 0:HD])
                    if s == NT - 1:
                        # pad-node slots get el = -1e30 so their alpha is 0
                        nc.vector.tensor_tensor(
                            out=pe[:, HD:HD + H], in0=cp[:, HD:HD + H],
                            in1=padt[:, 0:H], op=mybir.AluOpType.add)
                    else:
                        nc.scalar.copy(pe[:, HD:HD + H], cp[:, HD:HD + H])
                    if DR > HD + H:
                        nc.vector.memset(pe[:, HD + H:DR], 0.0)
                    nc.scalar.copy(er_sb[:, s, :], cp[:, HD + H:PR])
                    nc.sync.dma_start(piece[s * 128:(s + 1) * 128, :], pe[:])

                # ---- halo exchange ----
                nc.gpsimd.collective_compute(
                    "AllGather", mybir.AluOpType.bypass,
                    replica_groups=[list(range(NC))],
                    ins=[piece[:]], outs=[table[:]],
                )

                # ---- edge phase, one chunk at a time ----
                for (t0, G, Kc, col0) in chunks:
                    GK = G * Kc
                    gt = gtp.tile([128, GK, DR], bf16, tag="gt")
                    for j in range(GK):
                        nc.gpsimd.indirect_dma_start(
                            out=gt[:, j, :], out_offset=None, in_=table[:],
                            in_offset=bass.IndirectOffsetOnAxis(
                                ap=it[:, col0 + j:col0 + j + 1], axis=0),
                        )
                    # e = prelu(el_src + er_dst); alpha = exp(e)
                    e1 = wkp.tile([128, GK, H], f32, tag="e1")
                    nc.vector.tensor_tensor(
                        out=e1[:].rearrange("p (g k) h -> p g k h", g=G),
                        in0=gt[:, :, HD:HD + H].rearrange(
                            "p (g k) h -> p g k h", g=G),
                        in1=er_sb[:, t0:t0 + G, None, :].broadcast_to(
                            [128, G, Kc, H]),
                        op=mybir.AluOpType.add)
                    e2 = wkp.tile([128, GK, H], f32, tag="e2")
                    nc.scalar.activation(
                        e2[:].rearrange("p q h -> p (q h)"),
                        e1[:].rearrange("p q h -> p (q h)"),
                        mybir.ActivationFunctionType.Prelu, alpha=NEG_SLOPE)
                    alpha = wkp.tile([128, GK, H], bf16, tag="al")
                    nc.scalar.activation(
                        alpha[:].rearrange("p q h -> p (q h)"),
                        e2[:].rearrange("p q h -> p (q h)"),
                        mybir.ActivationFunctionType.Exp)
                    # denominators per (tile, head)
                    den = wkp.tile([128, G, H], f32, tag="den")
                    nc.vector.tensor_reduce(
                        out=den[:],
                        in_=alpha[:].rearrange("p (g k) h -> p g h k", g=G),
                        axis=mybir.AxisListType.X, op=mybir.AluOpType.add)
                    nc.vector.tensor_scalar_max(
                        den[:].rearrange("p g h -> p (g h)"),
                        den[:].rearrange("p g h -> p (g h)"), 1e-30)
                    rden = wkp.tile([128, G, H], f32, tag="rden")
                    nc.vector.reciprocal(
                        rden[:].rearrange("p g h -> p (g h)"),
                        den[:].rearrange("p g h -> p (g h)"))
                    # weighted messages, tree-reduced over edge slots
                    m = mmp.tile([128, GK, HD], bf16, tag="m")
                    nc.vector.tensor_tensor(
                        out=m[:].rearrange("p q (h d) -> p q h d", h=H),
                        in0=gt[:, :, 0:HD].rearrange(
                            "p q (h d) -> p q h d", h=H),
                        in1=alpha[:, :, :, None].broadcast_to([128, GK, H, D]),
                        op=mybir.AluOpType.mult)
                    m4 = m[:].rearrange("p (g k) f -> p g k f", g=G)
                    w = Kc
                    while w > 1:
                        h2 = w // 2
                        nc.vector.tensor_tensor(
                            out=m4[:, :, 0:h2, :], in0=m4[:, :, 0:h2, :],
                            in1=m4[:, :, h2:2 * h2, :],
                            op=mybir.AluOpType.add)
                        if w & 1:
                            nc.vector.tensor_tensor(
                                out=m4[:, :, 0, :], in0=m4[:, :, 0, :],
                                in1=m4[:, :, w - 1, :],
                                op=mybir.AluOpType.add)
                        w = h2
                    # normalize, bias, relu
                    o = wkp.tile([128, G, HD], f32, tag="o")
                    nc.vector.tensor_tensor(
                        out=o[:].rearrange("p g (h d) -> p g h d", h=H),
                        in0=m4[:, :, 0, :].rearrange(
                            "p g (h d) -> p g h d", h=H),
                        in1=rden[:, :, :, None].broadcast_to([128, G, H, D]),
                        op=mybir.AluOpType.mult)
                    nc.vector.tensor_tensor(
                        out=o[:], in0=o[:],
                        in1=b_t[l][:, None, :].broadcast_to([128, G, HD]),
                        op=mybir.AluOpType.add)
                    nc.vector.tensor_scalar_max(
                        o[:].rearrange("p g f -> p (g f)"),
                        o[:].rearrange("p g f -> p (g f)"), 0.0)
                    # transpose per tile into next layer's lhsT
                    for g in range(G):
                        tp = psb.tile([HD, 128], f32, tag="ps_b", space="PSUM")
                        nc.tensor.transpose(tp[:], o[:, g, :], ident[:])
                        nc.scalar.copy(
                            xt_next[:, (t0 + g) * 128:(t0 + g + 1) * 128],
                            tp[:])

            # ---- final FC ----
            for s in range(NT):
                fp = psa.tile([128, 93], f32, tag="ps_a", space="PSUM")
                nc.tensor.matmul(fp[:], lhsT=xts[3][:, s * 128:(s + 1) * 128],
                                 rhs=fct[:], start=True, stop=True)
                fo = pcp.tile([128, 93], f32, tag="fo")
                nc.vector.tensor_tensor(out=fo[:], in0=fp[:], in1=fcbt[:],
                                        op=mybir.AluOpType.add)
                nc.sync.dma_start(out93[s * 128:(s + 1) * 128, :], fo[:])

    nc.compile()
    return nc


def _prepare(inputs):
    import ml_dtypes
    bf16 = ml_dtypes.bfloat16

    src = np.ascontiguousarray(np.asarray(inputs["src"], dtype=np.int32))
    dst = np.ascontiguousarray(np.asarray(inputs["dst"], dtype=np.int32))
    feats = np.asarray(inputs["features"], dtype=np.float32)

    node_core, node_loc, chunks, C_tot, idx_arr = _preprocess(src, dst)

    ck = (chunks, C_tot)
    if ck not in _cache:
        _cache[ck] = _build(chunks, C_tot)
    nc = _cache[ck]

    pTl = []
    btl = []
    for l in range(3):
        W = np.asarray(inputs[f"W{l + 1}"], np.float32)
        al = np.asarray(inputs[f"al{l + 1}"], np.float32)
        ar = np.asarray(inputs[f"ar{l + 1}"], np.float32)
        b = np.asarray(inputs[f"b{l + 1}"], np.float32)
        P = _proj_matrix(W, al, ar)                          # [PR, Fin]
        pTl.append(np.ascontiguousarray(P.T.astype(bf16)))   # [Fin, PR]
        btl.append(np.ascontiguousarray(
            np.tile(b[None, :], (128, 1)).astype(np.float32)))
    fcw = np.asarray(inputs["fc_w"], np.float32)             # [93, 50]
    fcb = np.asarray(inputs["fc_b"], np.float32)
    fcT = np.ascontiguousarray(fcw.T.astype(bf16))           # [50, 93]
    fcb128 = np.ascontiguousarray(
        np.tile(fcb[None, :], (128, 1)).astype(np.float32))
    # last tile: partitions >= NPC - 48*128 (=106) are pad-node slots
    padmask = np.zeros((128, 4), np.float32)
    padmask[NPC - 48 * 128:, :] = -1e30

    in_maps = []
    for c in range(NC):
        xfm = np.zeros((25, NPCP), np.float32)
        sel = node_core == c
        xfm[:, node_loc[sel]] = feats[sel].T
        m = {"xin": xfm.astype(bf16), "idx": np.ascontiguousarray(idx_arr[c]),
             "fcT": fcT, "fcb": fcb128, "padm": padmask}
        for l in range(3):
            m[f"pT{l}"] = pTl[l]
            m[f"bias{l}"] = btl[l]
        in_maps.append(m)
    return nc, in_maps, node_core, node_loc


def _unshard(results, node_core, node_loc):
    out = np.zeros((N_NODES, 93), np.float32)
    for c in range(NC):
        o = np.asarray(results[c]["out93"], np.float32)      # [NPCP, 93]
        sel = node_core == c
        out[np.where(sel)[0]] = o[node_loc[sel]]
    return out


def kernel(**inputs):
    from concourse import bass_utils

    nc, in_maps, node_core, node_loc = _prepare(inputs)
    res = bass_utils.run_bass_kernel_spmd(nc, in_maps, core_ids=list(range(NC)))
    return _unshard(res.results, node_core, node_loc)


# revision 16
# speedup vs baseline: 1.0511x; 1.0511x over previous
"""Distributed 3-layer GAT + FC kernel for Trainium2 (8 NeuronCores).

Strategy (graph/data parallel, per the sharding hint):
  - Nodes are assigned to the 8 cores by in-degree rank interleaving
    (rank r -> core r%8, local slot r//8), so every core gets an almost
    identical degree profile and the SPMD program is shared.
  - Each core owns its nodes' incoming edges, laid out degree-bucketed:
    dst node = partition, incoming-edge slot = free-dim column.  One
    indirect DMA (128 rows, one per partition) gathers the source-node
    table rows for one edge-slot column.  Tiles (128 dst nodes) are
    grouped into chunks with a uniform, even slot count Kc so the edge
    math runs as a handful of wide DVE/ACT ops per chunk instead of
    per-head per-tile ops.
  - Per layer, each core projects its own nodes ([h | el | er] in one
    fused matmul; the el/er attention reductions are folded into the
    projection matrix host-side), then an AllGather replicates the
    bf16 [h | el] node table to every core (the halo exchange).
  - Edge softmax: the segment max is skipped (|e| <= ~10 on this data,
    exp cannot overflow) and the alpha normalization is applied after
    the weighted tree-reduction:
        out[n] = (sum_e exp(e_e) h[src_e]) / (sum_e exp(e_e)).
    Padding edge slots point at a pad-node table row whose el is set
    to -1e30, so exp() contributes exactly 0.
  - Table/gather/message datapath is bf16 (relative-error budget
    2e-2); the softmax path and final outputs are fp32.
  - Small weight tensors are replicated; the final FC stays node-local.
"""

import numpy as np

N_NODES = 50000
N_EDGES = 1000000
NC = 8
NPC = N_NODES // NC          # 6250 owned nodes per core
NT = 49                      # node tiles per core (128 nodes each)
NPCP = NT * 128              # 6272 padded nodes per core
NTOT = NPCP * NC             # 50176 table rows
PADROW = 6250                # core 0's first pad slot (el = -1e30)
NEG_SLOPE = 0.2

COLS_MAX = 128               # max G*Kc gathered per chunk
G_MAX = 3                    # max tiles per chunk

# (Fin, H, D) per GAT layer
LAYERS = [(25, 4, 10), (40, 4, 25), (100, 1, 50)]

_cache = {}


def _patch_tile_drain():
    """walrus in this toolchain rejects instructions carrying more than one
    semaphore wait; split the TileContext tail drain's waits onto
    single-wait NOPs."""
    import concourse.tile as tile_mod
    import concourse.mybir as mybir
    from concourse.vector_clock import ScopedClock

    if getattr(tile_mod.TileContext, "_drain_patched", False):
        return

    def _patched(self, tick_clock, wait_clock):
        nc = self.nc
        drain_inst = nc.sync.drain()
        wait_clock.add_sem_waits(
            drain_inst.ins, ScopedClock({None: tick_clock.global_clock})
        )
        si = drain_inst.ins.sync_info
        waits = list(si.on_wait or []) if si is not None else []
        if len(waits) > 1:
            si.on_wait.clear()
            bb = nc.cur_bb.bb
            assert bb.instructions[-1] is drain_inst.ins
            bb.instructions.pop()
            for w in waits:
                nop = nc.sync.nop(nofuse=True, hint="drain_wait_split")
                if nop.ins.sync_info is None:
                    nop.ins.sync_info = mybir.SyncInfo(on_wait=[w], on_update=[])
                else:
                    nop.ins.sync_info.on_wait.append(w)
            bb.add_instruction(drain_inst.ins)
        nc.all_engine_barrier()
        assert self.sems is not None
        popped = nc._tile_sem_poison_stack.pop()
        assert popped is self._sem_poison
        nc.clear_and_free_semaphores(list(self.sems.allocated().values()))
        nc.all_engine_barrier()

    tile_mod.TileContext._drain_and_barrier = _patched
    tile_mod.TileContext._drain_patched = True


def _preprocess(src, dst):
    """Node->core assignment, chunked degree layout, gather indices."""
    deg = np.bincount(dst, minlength=N_NODES)
    rank = np.argsort(-deg, kind="stable")
    node_core = np.empty(N_NODES, np.int64)
    node_loc = np.empty(N_NODES, np.int64)
    node_core[rank] = np.arange(N_NODES) % NC
    node_loc[rank] = np.arange(N_NODES) // NC
    glob = node_core * NPCP + node_loc

    loc_deg = np.zeros((NC, NPCP), np.int64)
    loc_deg[node_core, node_loc] = deg
    K_t = [int(loc_deg[:, t * 128:(t + 1) * 128].max()) for t in range(NT)]

    # group tiles into chunks with a shared, even Kc (K_t is non-increasing)
    chunks = []                      # (t0, G, Kc, col0)
    t = 0
    col = 0
    while t < NT:
        Kc = max(1, K_t[t])
        G = 1
        while (t + G < NT and G < G_MAX and (G + 1) * Kc <= COLS_MAX):
            G += 1
        chunks.append((t, G, Kc, col))
        col += G * Kc
        t += G
    C_tot = col

    idx_arr = np.full((NC, 128, C_tot), PADROW, np.int32)
    key = node_core[dst] * NPCP + node_loc[dst]
    eorder = np.argsort(key, kind="stable")
    ksorted = key[eorder]
    starts = np.searchsorted(ksorted, np.arange(NC * NPCP))
    ends = np.searchsorted(ksorted, np.arange(NC * NPCP) + 1)
    gsorted = glob[src[eorder]]
    for c in range(NC):
        base = c * NPCP
        for (t0, G, Kc, col0) in chunks:
            for g in range(G):
                tt = t0 + g
                cb = col0 + g * Kc
                for u in range(128):
                    loc = base + tt * 128 + u
                    s0, s1 = starts[loc], ends[loc]
                    if s1 > s0:
                        idx_arr[c, u, cb:cb + (s1 - s0)] = gsorted[s0:s1]
    return node_core, node_loc, tuple(chunks), C_tot, idx_arr


def _proj_matrix(W, al, ar):
    """P = [W; L^T W; R^T W] so that x @ P.T = [h | el | er] per node."""
    H, D = al.shape
    HD = H * D
    L = np.zeros((HD, H), np.float32)
    R = np.zeros((HD, H), np.float32)
    for h in range(H):
        L[h * D:(h + 1) * D, h] = al[h]
        R[h * D:(h + 1) * D, h] = ar[h]
    return np.vstack([W, L.T @ W, R.T @ W]).astype(np.float32)


def _build(chunks, C_tot):
    import concourse.bass as bass
    import concourse.bacc as bacc
    import concourse.mybir as mybir
    from concourse.tile import TileContext
    from concourse.masks import make_identity

    _patch_tile_drain()

    nc = bacc.Bacc("TRN2", target_bir_lowering=False, debug=False, num_devices=NC)
    f32 = mybir.dt.float32
    bf16 = mybir.dt.bfloat16

    # ---- external inputs ----
    xin = nc.dram_tensor("xin", [25, NPCP], bf16, kind="ExternalInput")
    idx = nc.dram_tensor("idx", [128, C_tot], mybir.dt.int32, kind="ExternalInput")
    pTs = []
    bts = []
    for l, (Fin, H, D) in enumerate(LAYERS):
        pTs.append(nc.dram_tensor(f"pT{l}", [Fin, H * D + 2 * H], bf16,
                                  kind="ExternalInput"))
        bts.append(nc.dram_tensor(f"bias{l}", [128, H * D], f32,
                                  kind="ExternalInput"))
    fcT = nc.dram_tensor("fcT", [50, 93], bf16, kind="ExternalInput")
    fcb = nc.dram_tensor("fcb", [128, 93], f32, kind="ExternalInput")
    padm = nc.dram_tensor("padm", [128, 4], f32, kind="ExternalInput")
    out93 = nc.dram_tensor("out93", [NPCP, 93], f32, kind="ExternalOutput")

    # ---- internal DRAM: halo pieces and gathered tables (bf16) ----
    pieces = []
    tables = []
    DRs = []
    for l, (Fin, H, D) in enumerate(LAYERS):
        DR = H * D + H
        DR += DR & 1                     # even row width (4B-aligned rows)
        DRs.append(DR)
        pieces.append(nc.dram_tensor(f"piece{l}", [NPCP, DR], bf16,
                                     kind="Internal"))
        tables.append(nc.dram_tensor(f"table{l}", [NTOT, DR], bf16,
                                     kind="Internal", addr_space="Shared"))

    with TileContext(nc) as tc:
        with (
            tc.tile_pool(name="const", bufs=1) as cpool,
            tc.tile_pool(name="pc", bufs=3) as pcp,
            tc.tile_pool(name="gt", bufs=3) as gtp,
            tc.tile_pool(name="mm", bufs=2) as mmp,
            tc.tile_pool(name="wk", bufs=3) as wkp,
            tc.tile_pool(name="psa", bufs=4, space="PSUM") as psa,
            tc.tile_pool(name="psb", bufs=4, space="PSUM") as psb,
        ):
            ident = cpool.tile([128, 128], f32, tag="ident")
            make_identity(nc, ident[:])
            it = cpool.tile([128, C_tot], mybir.dt.int32, tag="idx")
            nc.sync.dma_start(it[:], idx[:])
            pt_t = []
            b_t = []
            for l, (Fin, H, D) in enumerate(LAYERS):
                HD = H * D
                p = cpool.tile([Fin, HD + 2 * H], bf16, tag=f"pt{l}")
                nc.sync.dma_start(p[:], pTs[l][:])
                pt_t.append(p)
                b = cpool.tile([128, HD], f32, tag=f"b{l}")
                nc.sync.dma_start(b[:], bts[l][:])
                b_t.append(b)
            fct = cpool.tile([50, 93], bf16, tag="fct")
            nc.sync.dma_start(fct[:], fcT[:])
            fcbt = cpool.tile([128, 93], f32, tag="fcbt")
            nc.sync.dma_start(fcbt[:], fcb[:])
            padt = cpool.tile([128, 4], f32, tag="padt")
            nc.sync.dma_start(padt[:], padm[:])
            ers = [cpool.tile([128, NT, LAYERS[l][1]], bf16, tag=f"er{l}",
                              name=f"er{l}") for l in range(3)]
            xt0 = cpool.tile([25, NPCP], bf16, tag="xt0")
            nc.sync.dma_start(xt0[:], xin[:])
            xts = [xt0,
                   cpool.tile([40, NPCP], bf16, tag="xt1", name="xt1"),
                   cpool.tile([100, NPCP], bf16, tag="xt2", name="xt2"),
                   cpool.tile([50, NPCP], bf16, tag="xt3", name="xt3")]

            for l, (Fin, H, D) in enumerate(LAYERS):
                HD = H * D
                DR = DRs[l]
                PR = HD + 2 * H
                xt = xts[l]
                xt_next = xts[l + 1]
                table = tables[l]
                piece = pieces[l]
                er_sb = ers[l]

                # ---- projection of own nodes -> piece rows [h | el] ----
                for s in range(NT):
                    cp = psa.tile([128, PR], f32, tag="ps_a", space="PSUM")
                    nc.tensor.matmul(cp[:], lhsT=xt[:, s * 128:(s + 1) * 128],
                                     rhs=pt_t[l][:], start=True, stop=True)
                    pe = pcp.tile([128, DR], bf16, tag="pe")
                    nc.scalar.copy(pe[:, 0:HD], cp[:, 0:HD])
                    if s == NT - 1:
                        # pad-node slots get el = -1e30 so their alpha is 0
                        nc.vector.tensor_tensor(
                            out=pe[:, HD:HD + H], in0=cp[:, HD:HD + H],
                            in1=padt[:, 0:H], op=mybir.AluOpType.add)
                    else:
                        nc.scalar.copy(pe[:, HD:HD + H], cp[:, HD:HD + H])
                    if DR > HD + H:
                        nc.vector.memset(pe[:, HD + H:DR], 0.0)
                    nc.scalar.copy(er_sb[:, s, :], cp[:, HD + H:PR])
                    nc.sync.dma_start(piece[s * 128:(s + 1) * 128, :], pe[:])

                # ---- halo exchange ----
                nc.gpsimd.collective_compute(
                    "AllGather", mybir.AluOpType.bypass,
                    replica_groups=[list(range(NC))],
                    ins=[piece[:]], outs=[table[:]],
                )

                # ---- edge phase, one chunk at a time ----
                for (t0, G, Kc, col0) in chunks:
                    GK = G * Kc
                    gt = gtp.tile([128, GK, DR], bf16, tag="gt")
                    for j in range(GK):
                        nc.gpsimd.indirect_dma_start(
                            out=gt[:, j, :], out_offset=None, in_=table[:],
                            in_offset=bass.IndirectOffsetOnAxis(
                                ap=it[:, col0 + j:col0 + j + 1], axis=0),
                        )
                    # e = prelu(el_src + er_dst); alpha = exp(e)
                    e1 = wkp.tile([128, GK, H], f32, tag="e1")
                    nc.vector.tensor_tensor(
                        out=e1[:].rearrange("p (g k) h -> p g k h", g=G),
                        in0=gt[:, :, HD:HD + H].rearrange(
                            "p (g k) h -> p g k h", g=G),
                        in1=er_sb[:, t0:t0 + G, None, :].broadcast_to(
                            [128, G, Kc, H]),
                        op=mybir.AluOpType.add)
                    e2 = wkp.tile([128, GK, H], f32, tag="e2")
                    nc.scalar.activation(
                        e2[:].rearrange("p q h -> p (q h)"),
                        e1[:].rearrange("p q h -> p (q h)"),
                        mybir.ActivationFunctionType.Prelu, alpha=NEG_SLOPE)
                    alpha = wkp.tile([128, GK, H], bf16, tag="al")
                    nc.scalar.activation(
                        alpha[:].rearrange("p q h -> p (q h)"),
                        e2[:].rearrange("p q h -> p (q h)"),
                        mybir.ActivationFunctionType.Exp)
                    # denominators per (tile, head)
                    den = wkp.tile([128, G, H], f32, tag="den")
                    nc.vector.tensor_reduce(
                        out=den[:],
                        in_=alpha[:].rearrange("p (g k) h -> p g h k", g=G),
                        axis=mybir.AxisListType.X, op=mybir.AluOpType.add)
                    nc.vector.tensor_scalar_max(
                        den[:].rearrange("p g h -> p (g h)"),
                        den[:].rearrange("p g h -> p (g h)"), 1e-30)
                    rden = wkp.tile([128, G, H], f32, tag="rden")
                    nc.vector.reciprocal(
                        rden[:].rearrange("p g h -> p (g h)"),
                        den[:].rearrange("p g h -> p (g h)"))
                    # weighted messages, tree-reduced over edge slots
                    m = mmp.tile([128, GK, HD], bf16, tag="m")
                    nc.vector.tensor_tensor(
                        out=m[:].rearrange("p q (h d) -> p q h d", h=H),
                        in0=gt[:, :, 0:HD].rearrange(
                            "p q (h d) -> p q h d", h=H),
                        in1=alpha[:, :, :, None].broadcast_to([128, GK, H, D]),
                        op=mybir.AluOpType.mult)
                    m4 = m[:].rearrange("p (g k) f -> p g k f", g=G)
                    w = Kc
                    while w > 1:
                        h2 = w // 2
                        nc.vector.tensor_tensor(
                            out=m4[:, :, 0:h2, :], in0=m4[:, :, 0:h2, :],
                            in1=m4[:, :, h2:2 * h2, :],
                            op=mybir.AluOpType.add)
                        if w & 1:
                            nc.vector.tensor_tensor(
                                out=m4[:, :, 0, :], in0=m4[:, :, 0, :],
                                in1=m4[:, :, w - 1, :],
                                op=mybir.AluOpType.add)
                        w = h2
                    # normalize, bias, relu
                    o = wkp.tile([128, G, HD], f32, tag="o")
                    nc.vector.tensor_tensor(
                        out=o[:].rearrange("p g (h d) -> p g h d", h=H),
                        in0=m4[:, :, 0, :].rearrange(
                            "p g (h d) -> p g h d", h=H),
                        in1=rden[:, :, :, None].broadcast_to([128, G, H, D]),
                        op=mybir.AluOpType.mult)
                    nc.vector.tensor_tensor(
                        out=o[:], in0=o[:],
                        in1=b_t[l][:, None, :].broadcast_to([128, G, HD]),
                        op=mybir.AluOpType.add)
                    nc.vector.tensor_scalar_max(
                        o[:].rearrange("p g f -> p (g f)"),
                        o[:].rearrange("p g f -> p (g f)"), 0.0)
                    # transpose per tile into next layer's lhsT
                    for g in range(G):
                        tp = psb.tile([HD, 128], f32, tag="ps_b", space="PSUM")
                        nc.tensor.transpose(tp[:], o[:, g, :], ident[:])
                        nc.scalar.copy(
                            xt_next[:, (t0 + g) * 128:(t0 + g + 1) * 128],
                            tp[:])

            # ---- final FC ----
            for s in range(NT):
                fp = psa.tile([128, 93], f32, tag="ps_a", space="PSUM")
                nc.tensor.matmul(fp[:], lhsT=xts[3][:, s * 128:(s + 1) * 128],
                                 rhs=fct[:], start=True, stop=True)
                fo = pcp.tile([128, 93], f32, tag="fo")
                nc.vector.tensor_tensor(out=fo[:], in0=fp[:], in1=fcbt[:],
                                        op=mybir.AluOpType.add)
                nc.sync.dma_start(out93[s * 128:(s + 1) * 128, :], fo[:])

    nc.compile()
    return nc


def _prepare(inputs):
    import ml_dtypes
    bf16 = ml_dtypes.bfloat16

    src = np.ascontiguousarray(np.asarray(inputs["src"], dtype=np.int32))
    dst = np.ascontiguousarray(np.asarray(inputs["dst"], dtype=np.int32))
    feats = np.asarray(inputs["features"], dtype=np.float32)

    node_core, node_loc, chunks, C_tot, idx_arr = _preprocess(src, dst)

    ck = (chunks, C_tot)
    if ck not in _cache:
        _cache[ck] = _build(chunks, C_tot)
    nc = _cache[ck]

    pTl = []
    btl = []
    for l in range(3):
        W = np.asarray(inputs[f"W{l + 1}"], np.float32)
        al = np.asarray(inputs[f"al{l + 1}"], np.float32)
        ar = np.asarray(inputs[f"ar{l + 1}"], np.float32)
        b = np.asarray(inputs[f"b{l + 1}"], np.float32)
        P = _proj_matrix(W, al, ar)                          # [PR, Fin]
        pTl.append(np.ascontiguousarray(P.T.astype(bf16)))   # [Fin, PR]
        btl.append(np.ascontiguousarray(
            np.tile(b[None, :], (128, 1)).astype(np.float32)))
    fcw = np.asarray(inputs["fc_w"], np.float32)             # [93, 50]
    fcb = np.asarray(inputs["fc_b"], np.float32)
    fcT = np.ascontiguousarray(fcw.T.astype(bf16))           # [50, 93]
    fcb128 = np.ascontiguousarray(
        np.tile(fcb[None, :], (128, 1)).astype(np.float32))
    # last tile: partitions >= NPC - 48*128 (=106) are pad-node slots
    padmask = np.zeros((128, 4), np.float32)
    padmask[NPC - 48 * 128:, :] = -1e30

    in_maps = []
    for c in range(NC):
        xfm = np.zeros((25, NPCP), np.float32)
        sel = node_core == c
        xfm[:, node_loc[sel]] = feats[sel].T
        m = {"xin": xfm.astype(bf16), "idx": np.ascontiguousarray(idx_arr[c]),
             "fcT": fcT, "fcb": fcb128, "padm": padmask}
        for l in range(3):
            m[f"pT{l}"] = pTl[l]
            m[f"bias{l}"] = btl[l]
        in_maps.append(m)
    return nc, in_maps, node_core, node_loc


def _unshard(results, node_core, node_loc):
    out = np.zeros((N_NODES, 93), np.float32)
    for c in range(NC):
        o = np.asarray(results[c]["out93"], np.float32)      # [NPCP, 93]
        sel = node_core == c
        out[np.where(sel)[0]] = o[node_loc[sel]]
    return out


def kernel(**inputs):
    from concourse import bass_utils

    nc, in_maps, node_core, node_loc = _prepare(inputs)
    res = bass_utils.run_bass_kernel_spmd(nc, in_maps, core_ids=list(range(NC)))
    return _unshard(res.results, node_core, node_loc)


# revision 18
# speedup vs baseline: 1.0841x; 1.0314x over previous
"""Distributed 3-layer GAT + FC kernel for Trainium2 (8 NeuronCores).

Strategy (graph/data parallel, per the sharding hint):
  - Nodes are assigned to the 8 cores by in-degree rank interleaving
    (rank r -> core r%8, local slot r//8), so every core gets an almost
    identical degree profile and the SPMD program is shared.
  - Each core owns its nodes' incoming edges, laid out degree-bucketed:
    dst node = partition, incoming-edge slot = free-dim column.  One
    indirect DMA (128 rows, one per partition) gathers the source-node
    table rows for one edge-slot column.  Tiles (128 dst nodes) are
    grouped into chunks with a uniform slot count Kc so the edge math
    runs as a few wide DVE/ACT ops per chunk.
  - Per layer, each core projects its own nodes ([h | el | er] in one
    fused matmul; the el/er attention reductions are folded into the
    projection matrix host-side), then an AllGather replicates the
    bf16 [h | el] node table to every core (the halo exchange).
    The projection of layer l+1 (and the final FC) is emitted INSIDE
    layer l's edge loop, right after each chunk produces its activation
    tiles, so PE/ACT/DMA run it concurrently with the remaining
    gathers and the AllGather launches right after the last gather.
  - Edge softmax: the segment max is skipped (|e| <= ~10 on this data,
    exp cannot overflow) and the alpha normalization is applied after
    the weighted tree-reduction:
        out[n] = (sum_e exp(e_e) h[src_e]) / (sum_e exp(e_e)).
    Padding edge slots point at a pad-node table row whose el is set
    to -1e30, so exp() contributes exactly 0.
  - Table/gather/message datapath is bf16 (relative-error budget
    2e-2); the softmax path and final outputs are fp32.
  - Small weight tensors are replicated; the final FC stays node-local.
"""

import numpy as np

N_NODES = 50000
N_EDGES = 1000000
NC = 8
NPC = N_NODES // NC          # 6250 owned nodes per core
NT = 49                      # node tiles per core (128 nodes each)
NPCP = NT * 128              # 6272 padded nodes per core
NTOT = NPCP * NC             # 50176 table rows
PADROW = 6250                # core 0's first pad slot (el = -1e30)
NEG_SLOPE = 0.2

COLS_MAX = 128               # max G*Kc gathered per chunk
G_MAX = 2                    # max tiles per chunk

# (Fin, H, D) per GAT layer
LAYERS = [(25, 4, 10), (40, 4, 25), (100, 1, 50)]

_cache = {}


def _patch_tile_drain():
    """walrus in this toolchain rejects instructions carrying more than one
    semaphore wait; split the TileContext tail drain's waits onto
    single-wait NOPs."""
    import concourse.tile as tile_mod
    import concourse.mybir as mybir
    from concourse.vector_clock import ScopedClock

    if getattr(tile_mod.TileContext, "_drain_patched", False):
        return

    def _patched(self, tick_clock, wait_clock):
        nc = self.nc
        drain_inst = nc.sync.drain()
        wait_clock.add_sem_waits(
            drain_inst.ins, ScopedClock({None: tick_clock.global_clock})
        )
        si = drain_inst.ins.sync_info
        waits = list(si.on_wait or []) if si is not None else []
        if len(waits) > 1:
            si.on_wait.clear()
            bb = nc.cur_bb.bb
            assert bb.instructions[-1] is drain_inst.ins
            bb.instructions.pop()
            for w in waits:
                nop = nc.sync.nop(nofuse=True, hint="drain_wait_split")
                if nop.ins.sync_info is None:
                    nop.ins.sync_info = mybir.SyncInfo(on_wait=[w], on_update=[])
                else:
                    nop.ins.sync_info.on_wait.append(w)
            bb.add_instruction(drain_inst.ins)
        nc.all_engine_barrier()
        assert self.sems is not None
        popped = nc._tile_sem_poison_stack.pop()
        assert popped is self._sem_poison
        nc.clear_and_free_semaphores(list(self.sems.allocated().values()))
        nc.all_engine_barrier()

    tile_mod.TileContext._drain_and_barrier = _patched
    tile_mod.TileContext._drain_patched = True


def _preprocess(src, dst):
    """Node->core assignment, chunked degree layout, gather indices."""
    deg = np.bincount(dst, minlength=N_NODES)
    rank = np.argsort(-deg, kind="stable")
    node_core = np.empty(N_NODES, np.int64)
    node_loc = np.empty(N_NODES, np.int64)
    node_core[rank] = np.arange(N_NODES) % NC
    node_loc[rank] = np.arange(N_NODES) // NC
    glob = node_core * NPCP + node_loc

    loc_deg = np.zeros((NC, NPCP), np.int64)
    loc_deg[node_core, node_loc] = deg
    K_t = [int(loc_deg[:, t * 128:(t + 1) * 128].max()) for t in range(NT)]

    # group tiles into chunks with a shared Kc (K_t is non-increasing)
    chunks = []                      # (t0, G, Kc, col0)
    t = 0
    col = 0
    while t < NT:
        Kc = max(1, K_t[t])
        G = 1
        while (t + G < NT and G < G_MAX and (G + 1) * Kc <= COLS_MAX):
            G += 1
        chunks.append((t, G, Kc, col))
        col += G * Kc
        t += G
    C_tot = col

    idx_arr = np.full((NC, 128, C_tot), PADROW, np.int32)
    key = node_core[dst] * NPCP + node_loc[dst]
    eorder = np.argsort(key, kind="stable")
    ksorted = key[eorder]
    starts = np.searchsorted(ksorted, np.arange(NC * NPCP))
    ends = np.searchsorted(ksorted, np.arange(NC * NPCP) + 1)
    gsorted = glob[src[eorder]]
    for c in range(NC):
        base = c * NPCP
        for (t0, G, Kc, col0) in chunks:
            for g in range(G):
                tt = t0 + g
                cb = col0 + g * Kc
                for u in range(128):
                    loc = base + tt * 128 + u
                    s0, s1 = starts[loc], ends[loc]
                    if s1 > s0:
                        idx_arr[c, u, cb:cb + (s1 - s0)] = gsorted[s0:s1]
    return node_core, node_loc, tuple(chunks), C_tot, idx_arr


def _proj_matrix(W, al, ar):
    """P = [W; L^T W; R^T W] so that x @ P.T = [h | el | er] per node."""
    H, D = al.shape
    HD = H * D
    L = np.zeros((HD, H), np.float32)
    R = np.zeros((HD, H), np.float32)
    for h in range(H):
        L[h * D:(h + 1) * D, h] = al[h]
        R[h * D:(h + 1) * D, h] = ar[h]
    return np.vstack([W, L.T @ W, R.T @ W]).astype(np.float32)


def _build(chunks, C_tot):
    import concourse.bass as bass
    import concourse.bacc as bacc
    import concourse.mybir as mybir
    from concourse.tile import TileContext
    from concourse.masks import make_identity

    _patch_tile_drain()

    nc = bacc.Bacc("TRN2", target_bir_lowering=False, debug=False, num_devices=NC)
    f32 = mybir.dt.float32
    bf16 = mybir.dt.bfloat16

    # ---- external inputs ----
    xin = nc.dram_tensor("xin", [25, NPCP], bf16, kind="ExternalInput")
    idx = nc.dram_tensor("idx", [128, C_tot], mybir.dt.int32, kind="ExternalInput")
    pTs = []
    bts = []
    for l, (Fin, H, D) in enumerate(LAYERS):
        pTs.append(nc.dram_tensor(f"pT{l}", [Fin, H * D + 2 * H], bf16,
                                  kind="ExternalInput"))
        bts.append(nc.dram_tensor(f"bias{l}", [128, H * D], f32,
                                  kind="ExternalInput"))
    fcT = nc.dram_tensor("fcT", [50, 93], bf16, kind="ExternalInput")
    fcb = nc.dram_tensor("fcb", [128, 93], f32, kind="ExternalInput")
    padm = nc.dram_tensor("padm", [128, 4], f32, kind="ExternalInput")
    out93 = nc.dram_tensor("out93", [NPCP, 93], f32, kind="ExternalOutput")

    # ---- internal DRAM: halo pieces and gathered tables (bf16) ----
    pieces = []
    tables = []
    DRs = []
    for l, (Fin, H, D) in enumerate(LAYERS):
        DR = H * D + H
        DR += DR & 1                     # even row width (4B-aligned rows)
        DRs.append(DR)
        pieces.append(nc.dram_tensor(f"piece{l}", [NPCP, DR], bf16,
                                     kind="Internal"))
        tables.append(nc.dram_tensor(f"table{l}", [NTOT, DR], bf16,
                                     kind="Internal", addr_space="Shared"))

    with TileContext(nc) as tc:
        with (
            tc.tile_pool(name="const", bufs=1) as cpool,
            tc.tile_pool(name="pc", bufs=3) as pcp,
            tc.tile_pool(name="gt", bufs=3) as gtp,
            tc.tile_pool(name="mm", bufs=2) as mmp,
            tc.tile_pool(name="wk", bufs=3) as wkp,
            tc.tile_pool(name="psa", bufs=4, space="PSUM") as psa,
            tc.tile_pool(name="psb", bufs=4, space="PSUM") as psb,
        ):
            ident = cpool.tile([128, 128], f32, tag="ident")
            make_identity(nc, ident[:])
            it = cpool.tile([128, C_tot], mybir.dt.int32, tag="idx")
            nc.sync.dma_start(it[:], idx[:])
            pt_t = []
            b_t = []
            for l, (Fin, H, D) in enumerate(LAYERS):
                HD = H * D
                p = cpool.tile([Fin, HD + 2 * H], bf16, tag=f"pt{l}")
                nc.sync.dma_start(p[:], pTs[l][:])
                pt_t.append(p)
                b = cpool.tile([128, HD], f32, tag=f"b{l}")
                nc.sync.dma_start(b[:], bts[l][:])
                b_t.append(b)
            fct = cpool.tile([50, 93], bf16, tag="fct")
            nc.sync.dma_start(fct[:], fcT[:])
            fcbt = cpool.tile([128, 93], f32, tag="fcbt")
            nc.sync.dma_start(fcbt[:], fcb[:])
            padt = cpool.tile([128, 4], f32, tag="padt")
            nc.sync.dma_start(padt[:], padm[:])
            ers = [cpool.tile([128, NT, LAYERS[l][1]], bf16, tag=f"er{l}",
                              name=f"er{l}") for l in range(3)]
            xt0 = cpool.tile([25, NPCP], bf16, tag="xt0")
            nc.sync.dma_start(xt0[:], xin[:])
            xts = [xt0,
                   cpool.tile([40, NPCP], bf16, tag="xt1", name="xt1"),
                   cpool.tile([100, NPCP], bf16, tag="xt2", name="xt2"),
                   cpool.tile([50, NPCP], bf16, tag="xt3", name="xt3")]

            def emit_proj(l, tiles):
                """Project own nodes of `tiles` for layer l: [h|el] piece rows
                plus the er table kept in SBUF."""
                Fin, H, D = LAYERS[l]
                HD = H * D
                PR = HD + 2 * H
                DR = DRs[l]
                for s in tiles:
                    cp = psa.tile([128, PR], f32, tag="ps_a", space="PSUM")
                    nc.tensor.matmul(cp[:],
                                     lhsT=xts[l][:, s * 128:(s + 1) * 128],
                                     rhs=pt_t[l][:], start=True, stop=True)
                    pe = pcp.tile([128, DR], bf16, tag="pe")
                    nc.scalar.copy(pe[:, 0:HD], cp[:, 0:HD])
                    if s == NT - 1:
                        # pad-node slots get el = -1e30 so their alpha is 0
                        nc.vector.tensor_tensor(
                            out=pe[:, HD:HD + H], in0=cp[:, HD:HD + H],
                            in1=padt[:, 0:H], op=mybir.AluOpType.add)
                    else:
                        nc.scalar.copy(pe[:, HD:HD + H], cp[:, HD:HD + H])
                    if DR > HD + H:
                        nc.vector.memset(pe[:, HD + H:DR], 0.0)
                    nc.scalar.copy(ers[l][:, s, :], cp[:, HD + H:PR])
                    nc.sync.dma_start(pieces[l][s * 128:(s + 1) * 128, :],
                                      pe[:])

            def emit_fc(tiles):
                for s in tiles:
                    fp = psa.tile([128, 93], f32, tag="ps_a", space="PSUM")
                    nc.tensor.matmul(fp[:],
                                     lhsT=xts[3][:, s * 128:(s + 1) * 128],
                                     rhs=fct[:], start=True, stop=True)
                    fo = pcp.tile([128, 93], f32, tag="fo")
                    nc.vector.tensor_tensor(out=fo[:], in0=fp[:], in1=fcbt[:],
                                            op=mybir.AluOpType.add)
                    nc.sync.dma_start(out93[s * 128:(s + 1) * 128, :], fo[:])

            emit_proj(0, range(NT))
            nc.gpsimd.collective_compute(
                "AllGather", mybir.AluOpType.bypass,
                replica_groups=[list(range(NC))],
                ins=[pieces[0][:]], outs=[tables[0][:]],
            )

            for l, (Fin, H, D) in enumerate(LAYERS):
                HD = H * D
                DR = DRs[l]
                xt_next = xts[l + 1]
                table = tables[l]
                er_sb = ers[l]

                # ---- edge phase, one chunk at a time; the next layer's
                # projection (or the FC) is emitted per chunk so it runs
                # under the remaining gathers ----
                for (t0, G, Kc, col0) in chunks:
                    GK = G * Kc
                    gt = gtp.tile([128, GK, DR], bf16, tag="gt")
                    for j in range(GK):
                        nc.gpsimd.indirect_dma_start(
                            out=gt[:, j, :], out_offset=None, in_=table[:],
                            in_offset=bass.IndirectOffsetOnAxis(
                                ap=it[:, col0 + j:col0 + j + 1], axis=0),
                        )
                    # e = prelu(el_src + er_dst); alpha = exp(e)
                    e1 = wkp.tile([128, GK, H], f32, tag="e1")
                    nc.vector.tensor_tensor(
                        out=e1[:].rearrange("p (g k) h -> p g k h", g=G),
                        in0=gt[:, :, HD:HD + H].rearrange(
                            "p (g k) h -> p g k h", g=G),
                        in1=er_sb[:, t0:t0 + G, None, :].broadcast_to(
                            [128, G, Kc, H]),
                        op=mybir.AluOpType.add)
                    e2 = wkp.tile([128, GK, H], f32, tag="e2")
                    nc.scalar.activation(
                        e2[:].rearrange("p q h -> p (q h)"),
                        e1[:].rearrange("p q h -> p (q h)"),
                        mybir.ActivationFunctionType.Prelu, alpha=NEG_SLOPE)
                    alpha = wkp.tile([128, GK, H], bf16, tag="al")
                    nc.scalar.activation(
                        alpha[:].rearrange("p q h -> p (q h)"),
                        e2[:].rearrange("p q h -> p (q h)"),
                        mybir.ActivationFunctionType.Exp)
                    # denominators per (tile, head)
                    den = wkp.tile([128, G, H], f32, tag="den")
                    nc.vector.tensor_reduce(
                        out=den[:],
                        in_=alpha[:].rearrange("p (g k) h -> p g h k", g=G),
                        axis=mybir.AxisListType.X, op=mybir.AluOpType.add)
                    nc.vector.tensor_scalar_max(
                        den[:].rearrange("p g h -> p (g h)"),
                        den[:].rearrange("p g h -> p (g h)"), 1e-30)
                    rden = wkp.tile([128, G, H], f32, tag="rden")
                    nc.vector.reciprocal(
                        rden[:].rearrange("p g h -> p (g h)"),
                        den[:].rearrange("p g h -> p (g h)"))
                    # weighted messages, tree-reduced over edge slots
                    m = mmp.tile([128, GK, HD], bf16, tag="m")
                    nc.vector.tensor_tensor(
                        out=m[:].rearrange("p q (h d) -> p q h d", h=H),
                        in0=gt[:, :, 0:HD].rearrange(
                            "p q (h d) -> p q h d", h=H),
                        in1=alpha[:, :, :, None].broadcast_to([128, GK, H, D]),
                        op=mybir.AluOpType.mult)
                    m4 = m[:].rearrange("p (g k) f -> p g k f", g=G)
                    w = Kc
                    while w > 1:
                        h2 = w // 2
                        nc.vector.tensor_tensor(
                            out=m4[:, :, 0:h2, :], in0=m4[:, :, 0:h2, :],
                            in1=m4[:, :, h2:2 * h2, :],
                            op=mybir.AluOpType.add)
                        if w & 1:
                            nc.vector.tensor_tensor(
                                out=m4[:, :, 0, :], in0=m4[:, :, 0, :],
                                in1=m4[:, :, w - 1, :],
                                op=mybir.AluOpType.add)
                        w = h2
                    # normalize, bias, relu
                    o = wkp.tile([128, G, HD], f32, tag="o")
                    nc.vector.tensor_tensor(
                        out=o[:].rearrange("p g (h d) -> p g h d", h=H),
                        in0=m4[:, :, 0, :].rearrange(
                            "p g (h d) -> p g h d", h=H),
                        in1=rden[:, :, :, None].broadcast_to([128, G, H, D]),
                        op=mybir.AluOpType.mult)
                    nc.vector.tensor_tensor(
                        out=o[:], in0=o[:],
                        in1=b_t[l][:, None, :].broadcast_to([128, G, HD]),
                        op=mybir.AluOpType.add)
                    nc.vector.tensor_scalar_max(
                        o[:].rearrange("p g f -> p (g f)"),
                        o[:].rearrange("p g f -> p (g f)"), 0.0)
                    # transpose per tile into next layer's lhsT, then
                    # immediately project (or FC) those tiles
                    for g in range(G):
                        tp = psb.tile([HD, 128], f32, tag="ps_b", space="PSUM")
                        nc.tensor.transpose(tp[:], o[:, g, :], ident[:])
                        nc.scalar.copy(
                            xt_next[:, (t0 + g) * 128:(t0 + g + 1) * 128],
                            tp[:])
                    if l < 2:
                        emit_proj(l + 1, range(t0, t0 + G))
                    else:
                        emit_fc(range(t0, t0 + G))

                if l < 2:
                    nc.gpsimd.collective_compute(
                        "AllGather", mybir.AluOpType.bypass,
                        replica_groups=[list(range(NC))],
                        ins=[pieces[l + 1][:]], outs=[tables[l + 1][:]],
                    )

    nc.compile()
    return nc


def _prepare(inputs):
    import ml_dtypes
    bf16 = ml_dtypes.bfloat16

    src = np.ascontiguousarray(np.asarray(inputs["src"], dtype=np.int32))
    dst = np.ascontiguousarray(np.asarray(inputs["dst"], dtype=np.int32))
    feats = np.asarray(inputs["features"], dtype=np.float32)

    node_core, node_loc, chunks, C_tot, idx_arr = _preprocess(src, dst)

    ck = (chunks, C_tot)
    if ck not in _cache:
        _cache[ck] = _build(chunks, C_tot)
    nc = _cache[ck]

    pTl = []
    btl = []
    for l in range(3):
        W = np.asarray(inputs[f"W{l + 1}"], np.float32)
        al = np.asarray(inputs[f"al{l + 1}"], np.float32)
        ar = np.asarray(inputs[f"ar{l + 1}"], np.float32)
        b = np.asarray(inputs[f"b{l + 1}"], np.float32)
        P = _proj_matrix(W, al, ar)                          # [PR, Fin]
        pTl.append(np.ascontiguousarray(P.T.astype(bf16)))   # [Fin, PR]
        btl.append(np.ascontiguousarray(
            np.tile(b[None, :], (128, 1)).astype(np.float32)))
    fcw = np.asarray(inputs["fc_w"], np.float32)             # [93, 50]
    fcb = np.asarray(inputs["fc_b"], np.float32)
    fcT = np.ascontiguousarray(fcw.T.astype(bf16))           # [50, 93]
    fcb128 = np.ascontiguousarray(
        np.tile(fcb[None, :], (128, 1)).astype(np.float32))
    # last tile: partitions >= NPC - 48*128 (=106) are pad-node slots
    padmask = np.zeros((128, 4), np.float32)
    padmask[NPC - 48 * 128:, :] = -1e30

    in_maps = []
    for c in range(NC):
        xfm = np.zeros((25, NPCP), np.float32)
        sel = node_core == c
        xfm[:, node_loc[sel]] = feats[sel].T
        m = {"xin": xfm.astype(bf16), "idx": np.ascontiguousarray(idx_arr[c]),
             "fcT": fcT, "fcb": fcb128, "padm": padmask}
        for l in range(3):
            m[f"pT{l}"] = pTl[l]
            m[f"bias{l}"] = btl[l]
        in_maps.append(m)
    return nc, in_maps, node_core, node_loc


def _unshard(results, node_core, node_loc):
    out = np.zeros((N_NODES, 93), np.float32)
    for c in range(NC):
        o = np.asarray(results[c]["out93"], np.float32)      # [NPCP, 93]
        sel = node_core == c
        out[np.where(sel)[0]] = o[node_loc[sel]]
    return out


def kernel(**inputs):
    from concourse import bass_utils

    nc, in_maps, node_core, node_loc = _prepare(inputs)
    res = bass_utils.run_bass_kernel_spmd(nc, in_maps, core_ids=list(range(NC)))
    return _unshard(res.results, node_core, node_loc)


# revision 26
# speedup vs baseline: 1.0889x; 1.0044x over previous
"""Distributed 3-layer GAT + FC kernel for Trainium2 (8 NeuronCores).

Strategy (graph/data parallel, per the sharding hint):
  - Nodes are assigned to the 8 cores by in-degree rank interleaving
    (rank r -> core r%8, local slot r//8), so every core gets an almost
    identical degree profile and the SPMD program is shared.
  - Each core owns its nodes' incoming edges, laid out degree-bucketed:
    dst node = partition, incoming-edge slot = free-dim column.  One
    indirect DMA (128 rows, one per partition) gathers the source-node
    table rows for one edge-slot column.  Tiles (128 dst nodes) are
    grouped into chunks with a uniform slot count Kc so the edge math
    runs as a few wide DVE/ACT ops per chunk.
  - Per layer, each core projects its own nodes ([h | el | er] in one
    fused matmul; the el/er attention reductions are folded into the
    projection matrix host-side), then an AllGather replicates the
    bf16 [h | el] node table to every core (the halo exchange).
    The projection of layer l+1 (and the final FC) is emitted INSIDE
    layer l's edge loop, right after each chunk produces its activation
    tiles, so PE/ACT/DMA run it concurrently with the remaining
    gathers and the AllGather launches right after the last gather.
  - Edge softmax: the segment max is skipped (|e| <= ~10 on this data,
    exp cannot overflow) and the alpha normalization is applied after
    the weighted tree-reduction:
        out[n] = (sum_e exp(e_e) h[src_e]) / (sum_e exp(e_e)).
    Padding edge slots point at a pad-node table row whose el is set
    to -1e30, so exp() contributes exactly 0.
  - Table/gather/message datapath is bf16 (relative-error budget
    2e-2); the softmax path and final outputs are fp32.
  - Small weight tensors are replicated; the final FC stays node-local.
"""

import numpy as np

N_NODES = 50000
N_EDGES = 1000000
NC = 8
NPC = N_NODES // NC          # 6250 owned nodes per core
NT = 49                      # node tiles per core (128 nodes each)
NPCP = NT * 128              # 6272 padded nodes per core
NTOT = NPCP * NC             # 50176 table rows
PADROW = 6250                # core 0's first pad slot (el = -1e30)
NEG_SLOPE = 0.2

COLS_MAX = 128               # max G*Kc gathered per chunk
G_MAX = 2                    # max tiles per chunk

# (Fin, H, D) per GAT layer
LAYERS = [(25, 4, 10), (40, 4, 25), (100, 1, 50)]

_cache = {}


def _patch_tile_drain():
    """walrus in this toolchain rejects instructions carrying more than one
    semaphore wait; split the TileContext tail drain's waits onto
    single-wait NOPs."""
    import concourse.tile as tile_mod
    import concourse.mybir as mybir
    from concourse.vector_clock import ScopedClock

    if getattr(tile_mod.TileContext, "_drain_patched", False):
        return

    def _patched(self, tick_clock, wait_clock):
        nc = self.nc
        drain_inst = nc.sync.drain()
        wait_clock.add_sem_waits(
            drain_inst.ins, ScopedClock({None: tick_clock.global_clock})
        )
        si = drain_inst.ins.sync_info
        waits = list(si.on_wait or []) if si is not None else []
        if len(waits) > 1:
            si.on_wait.clear()
            bb = nc.cur_bb.bb
            assert bb.instructions[-1] is drain_inst.ins
            bb.instructions.pop()
            for w in waits:
                nop = nc.sync.nop(nofuse=True, hint="drain_wait_split")
                if nop.ins.sync_info is None:
                    nop.ins.sync_info = mybir.SyncInfo(on_wait=[w], on_update=[])
                else:
                    nop.ins.sync_info.on_wait.append(w)
            bb.add_instruction(drain_inst.ins)
        nc.all_engine_barrier()
        assert self.sems is not None
        popped = nc._tile_sem_poison_stack.pop()
        assert popped is self._sem_poison
        nc.clear_and_free_semaphores(list(self.sems.allocated().values()))
        nc.all_engine_barrier()

    tile_mod.TileContext._drain_and_barrier = _patched
    tile_mod.TileContext._drain_patched = True


def _preprocess(src, dst):
    """Node->core assignment, chunked degree layout, gather indices."""
    deg = np.bincount(dst, minlength=N_NODES)
    rank = np.argsort(-deg, kind="stable")
    node_core = np.empty(N_NODES, np.int64)
    node_loc = np.empty(N_NODES, np.int64)
    node_core[rank] = np.arange(N_NODES) % NC
    node_loc[rank] = np.arange(N_NODES) // NC

    loc_deg = np.zeros((NC, NPCP), np.int64)
    loc_deg[node_core, node_loc] = deg
    K_t = [int(loc_deg[:, t * 128:(t + 1) * 128].max()) for t in range(NT)]

    # group tiles into chunks with a shared Kc (K_t is non-increasing)
    chunks = []                      # (t0, G, Kc, col0)
    t = 0
    col = 0
    while t < NT:
        Kc = max(1, K_t[t])
        G = 1
        while (t + G < NT and G < G_MAX and (G + 1) * Kc <= COLS_MAX):
            G += 1
        chunks.append((t, G, Kc, col))
        col += G * Kc
        t += G
    C_tot = col

    # table layout: two contiguous core-major blocks split at the chunk
    # boundary SPLIT_T, so each block is one contiguous AllGather output
    split_t = max(t0 + G for (t0, G, _, _) in chunks if t0 + G <= 36)
    r1 = split_t * 128
    glob = np.where(node_loc < r1,
                    node_core * r1 + node_loc,
                    NC * r1 + node_core * (NPCP - r1) + (node_loc - r1))
    padrow = NC * r1 + (NPC - r1)        # core 0's first pad slot

    idx_arr = np.full((NC, 128, C_tot), padrow, np.int32)
    key = node_core[dst] * NPCP + node_loc[dst]
    eorder = np.argsort(key, kind="stable")
    ksorted = key[eorder]
    starts = np.searchsorted(ksorted, np.arange(NC * NPCP))
    ends = np.searchsorted(ksorted, np.arange(NC * NPCP) + 1)
    gsorted = glob[src[eorder]]
    for c in range(NC):
        base = c * NPCP
        for (t0, G, Kc, col0) in chunks:
            for g in range(G):
                tt = t0 + g
                cb = col0 + g * Kc
                for u in range(128):
                    loc = base + tt * 128 + u
                    s0, s1 = starts[loc], ends[loc]
                    if s1 > s0:
                        idx_arr[c, u, cb:cb + (s1 - s0)] = gsorted[s0:s1]
    return node_core, node_loc, tuple(chunks), C_tot, idx_arr


def _proj_matrix(W, al, ar):
    """P = [W; L^T W; R^T W] so that x @ P.T = [h | el | er] per node."""
    H, D = al.shape
    HD = H * D
    L = np.zeros((HD, H), np.float32)
    R = np.zeros((HD, H), np.float32)
    for h in range(H):
        L[h * D:(h + 1) * D, h] = al[h]
        R[h * D:(h + 1) * D, h] = ar[h]
    return np.vstack([W, L.T @ W, R.T @ W]).astype(np.float32)


def _build(chunks, C_tot):
    import concourse.bass as bass
    import concourse.bacc as bacc
    import concourse.mybir as mybir
    from concourse.tile import TileContext
    from concourse.masks import make_identity

    _patch_tile_drain()

    nc = bacc.Bacc("TRN2", target_bir_lowering=False, debug=False, num_devices=NC)
    f32 = mybir.dt.float32
    bf16 = mybir.dt.bfloat16

    # ---- external inputs ----
    xin = nc.dram_tensor("xin", [25, NPCP], bf16, kind="ExternalInput")
    idx = nc.dram_tensor("idx", [128, C_tot], mybir.dt.int32, kind="ExternalInput")
    pTs = []
    bts = []
    for l, (Fin, H, D) in enumerate(LAYERS):
        pTs.append(nc.dram_tensor(f"pT{l}", [Fin, H * D + 2 * H], bf16,
                                  kind="ExternalInput"))
        bts.append(nc.dram_tensor(f"bias{l}", [128, H * D], f32,
                                  kind="ExternalInput"))
    fcT = nc.dram_tensor("fcT", [50, 93], bf16, kind="ExternalInput")
    fcb = nc.dram_tensor("fcb", [128, 93], f32, kind="ExternalInput")
    padm = nc.dram_tensor("padm", [128, 4], f32, kind="ExternalInput")
    out93 = nc.dram_tensor("out93", [NPCP, 93], f32, kind="ExternalOutput")

    # ---- internal DRAM: halo pieces and gathered tables (bf16) ----
    pieces = []
    tables = []
    DRs = []
    for l, (Fin, H, D) in enumerate(LAYERS):
        DR = H * D + H
        DR += DR & 1                     # even row width (4B-aligned rows)
        DRs.append(DR)
        pieces.append(nc.dram_tensor(f"piece{l}", [NPCP, DR], bf16,
                                     kind="Internal"))
        tables.append(nc.dram_tensor(f"table{l}", [NTOT, DR], bf16,
                                     kind="Internal", addr_space="Shared"))

    with TileContext(nc) as tc:
        with (
            tc.tile_pool(name="const", bufs=1) as cpool,
            tc.tile_pool(name="pc", bufs=3) as pcp,
            tc.tile_pool(name="gt", bufs=3) as gtp,
            tc.tile_pool(name="mm", bufs=2) as mmp,
            tc.tile_pool(name="wk", bufs=3) as wkp,
            tc.tile_pool(name="psa", bufs=4, space="PSUM") as psa,
            tc.tile_pool(name="psb", bufs=4, space="PSUM") as psb,
        ):
            ident = cpool.tile([128, 128], f32, tag="ident")
            make_identity(nc, ident[:])
            it = cpool.tile([128, C_tot], mybir.dt.int32, tag="idx")
            nc.sync.dma_start(it[:], idx[:])
            pt_t = []
            b_t = []
            for l, (Fin, H, D) in enumerate(LAYERS):
                HD = H * D
                p = cpool.tile([Fin, HD + 2 * H], bf16, tag=f"pt{l}")
                nc.sync.dma_start(p[:], pTs[l][:])
                pt_t.append(p)
                b = cpool.tile([128, HD], f32, tag=f"b{l}")
                nc.sync.dma_start(b[:], bts[l][:])
                b_t.append(b)
            fct = cpool.tile([50, 93], bf16, tag="fct")
            nc.sync.dma_start(fct[:], fcT[:])
            fcbt = cpool.tile([128, 93], f32, tag="fcbt")
            nc.sync.dma_start(fcbt[:], fcb[:])
            padt = cpool.tile([128, 4], f32, tag="padt")
            nc.sync.dma_start(padt[:], padm[:])
            ers = [cpool.tile([128, NT, LAYERS[l][1]], bf16, tag=f"er{l}",
                              name=f"er{l}") for l in range(3)]
            xt0 = cpool.tile([25, NPCP], bf16, tag="xt0")
            nc.sync.dma_start(xt0[:], xin[:])
            xts = [xt0,
                   cpool.tile([40, NPCP], bf16, tag="xt1", name="xt1"),
                   cpool.tile([100, NPCP], bf16, tag="xt2", name="xt2"),
                   cpool.tile([50, NPCP], bf16, tag="xt3", name="xt3")]

            def emit_proj(l, tiles):
                """Project own nodes of `tiles` for layer l: [h|el] piece rows
                plus the er table kept in SBUF."""
                Fin, H, D = LAYERS[l]
                HD = H * D
                PR = HD + 2 * H
                DR = DRs[l]
                for s in tiles:
                    cp = psa.tile([128, PR], f32, tag="ps_a", space="PSUM")
                    nc.tensor.matmul(cp[:],
                                     lhsT=xts[l][:, s * 128:(s + 1) * 128],
                                     rhs=pt_t[l][:], start=True, stop=True)
                    pe = pcp.tile([128, DR], bf16, tag="pe")
                    if s == NT - 1:
                        # pad-node slots get el = -1e30 so their alpha is 0
                        nc.scalar.copy(pe[:, 0:HD], cp[:, 0:HD])
                        nc.vector.tensor_tensor(
                            out=pe[:, HD:HD + H], in0=cp[:, HD:HD + H],
                            in1=padt[:, 0:H], op=mybir.AluOpType.add)
                    else:
                        nc.scalar.copy(pe[:, 0:HD + H], cp[:, 0:HD + H])
                    if DR > HD + H:
                        nc.vector.memset(pe[:, HD + H:DR], 0.0)
                    nc.scalar.copy(ers[l][:, s, :], cp[:, HD + H:PR])
                    nc.sync.dma_start(pieces[l][s * 128:(s + 1) * 128, :],
                                      pe[:])

            def emit_fc(tiles):
                for s in tiles:
                    fp = psa.tile([128, 93], f32, tag="ps_a", space="PSUM")
                    nc.tensor.matmul(fp[:],
                                     lhsT=xts[3][:, s * 128:(s + 1) * 128],
                                     rhs=fct[:], start=True, stop=True)
                    fo = pcp.tile([128, 93], f32, tag="fo")
                    nc.vector.tensor_tensor(out=fo[:], in0=fp[:], in1=fcbt[:],
                                            op=mybir.AluOpType.add)
                    nc.sync.dma_start(out93[s * 128:(s + 1) * 128, :], fo[:])

            # split point for the pipelined halo exchange: the latest chunk
            # boundary at or before tile 36 (~3/4 of the piece rows go out
            # early, the small remainder right after the last projection)
            SPLIT_T = max(t0 + G for (t0, G, _, _) in chunks if t0 + G <= 36)
            R1 = SPLIT_T * 128

            def emit_ag(l, part):
                """AllGather one contiguous block of layer l's halo table."""
                if part == 0:
                    ins_ = pieces[l][0:R1, :]
                    outs_ = tables[l][0:NC * R1, :]
                else:
                    ins_ = pieces[l][R1:NPCP, :]
                    outs_ = tables[l][NC * R1:NTOT, :]
                nc.gpsimd.collective_compute(
                    "AllGather", mybir.AluOpType.bypass,
                    replica_groups=[list(range(NC))],
                    ins=[ins_], outs=[outs_],
                )

            emit_proj(0, range(NT))
            emit_ag(0, 0)
            emit_ag(0, 1)

            for l, (Fin, H, D) in enumerate(LAYERS):
                HD = H * D
                DR = DRs[l]
                xt_next = xts[l + 1]
                table = tables[l]
                er_sb = ers[l]

                # ---- edge phase, one chunk at a time; the next layer's
                # projection (or the FC) is emitted per chunk so it runs
                # under the remaining gathers ----
                for (t0, G, Kc, col0) in chunks:
                    GK = G * Kc
                    gt = gtp.tile([128, GK, DR], bf16, tag="gt")
                    for j in range(GK):
                        nc.gpsimd.indirect_dma_start(
                            out=gt[:, j, :], out_offset=None, in_=table[:],
                            in_offset=bass.IndirectOffsetOnAxis(
                                ap=it[:, col0 + j:col0 + j + 1], axis=0),
                        )
                    # e = prelu(el_src + er_dst); alpha = exp(e)
                    e1 = wkp.tile([128, GK, H], f32, tag="e1")
                    nc.vector.tensor_tensor(
                        out=e1[:].rearrange("p (g k) h -> p g k h", g=G),
                        in0=gt[:, :, HD:HD + H].rearrange(
                            "p (g k) h -> p g k h", g=G),
                        in1=er_sb[:, t0:t0 + G, None, :].broadcast_to(
                            [128, G, Kc, H]),
                        op=mybir.AluOpType.add)
                    e2 = wkp.tile([128, GK, H], f32, tag="e2")
                    nc.scalar.activation(
                        e2[:].rearrange("p q h -> p (q h)"),
                        e1[:].rearrange("p q h -> p (q h)"),
                        mybir.ActivationFunctionType.Prelu, alpha=NEG_SLOPE)
                    alpha = wkp.tile([128, GK, H], bf16, tag="al")
                    nc.scalar.activation(
                        alpha[:].rearrange("p q h -> p (q h)"),
                        e2[:].rearrange("p q h -> p (q h)"),
                        mybir.ActivationFunctionType.Exp)
                    # denominators per (tile, head)
                    den = wkp.tile([128, G, H], f32, tag="den")
                    nc.vector.tensor_reduce(
                        out=den[:],
                        in_=alpha[:].rearrange("p (g k) h -> p g h k", g=G),
                        axis=mybir.AxisListType.X, op=mybir.AluOpType.add)
                    nc.vector.tensor_scalar_max(
                        den[:].rearrange("p g h -> p (g h)"),
                        den[:].rearrange("p g h -> p (g h)"), 1e-30)
                    rden = wkp.tile([128, G, H], f32, tag="rden")
                    nc.vector.reciprocal(
                        rden[:].rearrange("p g h -> p (g h)"),
                        den[:].rearrange("p g h -> p (g h)"))
                    # weighted messages, tree-reduced over edge slots
                    m = mmp.tile([128, GK, HD], bf16, tag="m")
                    nc.vector.tensor_tensor(
                        out=m[:].rearrange("p q (h d) -> p q h d", h=H),
                        in0=gt[:, :, 0:HD].rearrange(
                            "p q (h d) -> p q h d", h=H),
                        in1=alpha[:, :, :, None].broadcast_to([128, GK, H, D]),
                        op=mybir.AluOpType.mult)
                    m4 = m[:].rearrange("p (g k) f -> p g k f", g=G)
                    w = Kc
                    while w > 1:
                        h2 = w // 2
                        nc.vector.tensor_tensor(
                            out=m4[:, :, 0:h2, :], in0=m4[:, :, 0:h2, :],
                            in1=m4[:, :, h2:2 * h2, :],
                            op=mybir.AluOpType.add)
                        if w & 1:
                            nc.vector.tensor_tensor(
                                out=m4[:, :, 0, :], in0=m4[:, :, 0, :],
                                in1=m4[:, :, w - 1, :],
                                op=mybir.AluOpType.add)
                        w = h2
                    # normalize, bias, relu
                    o = wkp.tile([128, G, HD], f32, tag="o")
                    nc.vector.tensor_tensor(
                        out=o[:].rearrange("p g (h d) -> p g h d", h=H),
                        in0=m4[:, :, 0, :].rearrange(
                            "p g (h d) -> p g h d", h=H),
                        in1=rden[:, :, :, None].broadcast_to([128, G, H, D]),
                        op=mybir.AluOpType.mult)
                    nc.vector.tensor_tensor(
                        out=o[:], in0=o[:],
                        in1=b_t[l][:, None, :].broadcast_to([128, G, HD]),
                        op=mybir.AluOpType.add)
                    nc.vector.tensor_scalar_max(
                        o[:].rearrange("p g f -> p (g f)"),
                        o[:].rearrange("p g f -> p (g f)"), 0.0)
                    # transpose per tile into next layer's lhsT, then
                    # immediately project (or FC) those tiles
                    for g in range(G):
                        tp = psb.tile([HD, 128], f32, tag="ps_b", space="PSUM")
                        nc.tensor.transpose(tp[:], o[:, g, :], ident[:])
                        nc.scalar.copy(
                            xt_next[:, (t0 + g) * 128:(t0 + g + 1) * 128],
                            tp[:])
                    if l < 2:
                        emit_proj(l + 1, range(t0, t0 + G))
                        # first half of the next halo exchange launches as
                        # soon as its piece rows exist, hiding its transfer
                        # under this layer's remaining gathers
                        if t0 + G == SPLIT_T:
                            emit_ag(l + 1, 0)
                    else:
                        emit_fc(range(t0, t0 + G))

                if l < 2:
                    emit_ag(l + 1, 1)

    nc.compile()
    return nc


def _prepare(inputs):
    import ml_dtypes
    bf16 = ml_dtypes.bfloat16

    src = np.ascontiguousarray(np.asarray(inputs["src"], dtype=np.int32))
    dst = np.ascontiguousarray(np.asarray(inputs["dst"], dtype=np.int32))
    feats = np.asarray(inputs["features"], dtype=np.float32)

    node_core, node_loc, chunks, C_tot, idx_arr = _preprocess(src, dst)

    ck = (chunks, C_tot)
    if ck not in _cache:
        _cache[ck] = _build(chunks, C_tot)
    nc = _cache[ck]

    pTl = []
    btl = []
    for l in range(3):
        W = np.asarray(inputs[f"W{l + 1}"], np.float32)
        al = np.asarray(inputs[f"al{l + 1}"], np.float32)
        ar = np.asarray(inputs[f"ar{l + 1}"], np.float32)
        b = np.asarray(inputs[f"b{l + 1}"], np.float32)
        P = _proj_matrix(W, al, ar)                          # [PR, Fin]
        pTl.append(np.ascontiguousarray(P.T.astype(bf16)))   # [Fin, PR]
        btl.append(np.ascontiguousarray(
            np.tile(b[None, :], (128, 1)).astype(np.float32)))
    fcw = np.asarray(inputs["fc_w"], np.float32)             # [93, 50]
    fcb = np.asarray(inputs["fc_b"], np.float32)
    fcT = np.ascontiguousarray(fcw.T.astype(bf16))           # [50, 93]
    fcb128 = np.ascontiguousarray(
        np.tile(fcb[None, :], (128, 1)).astype(np.float32))
    # last tile: partitions >= NPC - 48*128 (=106) are pad-node slots
    padmask = np.zeros((128, 4), np.float32)
    padmask[NPC - 48 * 128:, :] = -1e30

    in_maps = []
    for c in range(NC):
        xfm = np.zeros((25, NPCP), np.float32)
        sel = node_core == c
        xfm[:, node_loc[sel]] = feats[sel].T
        m = {"xin": xfm.astype(bf16), "idx": np.ascontiguousarray(idx_arr[c]),
             "fcT": fcT, "fcb": fcb128, "padm": padmask}
        for l in range(3):
            m[f"pT{l}"] = pTl[l]
            m[f"bias{l}"] = btl[l]
        in_maps.append(m)
    return nc, in_maps, node_core, node_loc


def _unshard(results, node_core, node_loc):
    out = np.zeros((N_NODES, 93), np.float32)
    for c in range(NC):
        o = np.asarray(results[c]["out93"], np.float32)      # [NPCP, 93]
        sel = node_core == c
        out[np.where(sel)[0]] = o[node_loc[sel]]
    return out


def kernel(**inputs):
    from concourse import bass_utils

    nc, in_maps, node_core, node_loc = _prepare(inputs)
    res = bass_utils.run_bass_kernel_spmd(nc, in_maps, core_ids=list(range(NC)))
    return _unshard(res.results, node_core, node_loc)


# revision 29
# speedup vs baseline: 1.1129x; 1.0221x over previous
"""Distributed 3-layer GAT + FC kernel for Trainium2 (8 NeuronCores).

Strategy (graph/data parallel, per the sharding hint):
  - Nodes are assigned to the 8 cores by in-degree rank interleaving
    (rank r -> core r%8, local slot r//8), so every core gets an almost
    identical degree profile and the SPMD program is shared.
  - Each core owns its nodes' incoming edges, laid out degree-bucketed:
    dst node = partition, incoming-edge slot = free-dim column.  One
    indirect DMA (128 rows, one per partition) gathers the source-node
    table rows for one edge-slot column.  Tiles (128 dst nodes) are
    grouped into chunks with a uniform slot count Kc so the edge math
    runs as a few wide DVE/ACT ops per chunk.
  - Per layer, each core projects its own nodes ([h | el | er] in one
    fused matmul; the el/er attention reductions are folded into the
    projection matrix host-side), then an AllGather replicates the
    bf16 [h | el] node table to every core (the halo exchange).
    The projection of layer l+1 (and the final FC) is emitted INSIDE
    layer l's edge loop, right after each chunk produces its activation
    tiles, so PE/ACT/DMA run it concurrently with the remaining
    gathers and the AllGather launches right after the last gather.
  - Edge softmax: the segment max is skipped (|e| <= ~10 on this data,
    exp cannot overflow) and the alpha normalization is applied after
    the weighted tree-reduction:
        out[n] = (sum_e exp(e_e) h[src_e]) / (sum_e exp(e_e)).
    Padding edge slots point at a pad-node table row whose el is set
    to -1e30, so exp() contributes exactly 0.
  - Table/gather/message datapath is bf16 (relative-error budget
    2e-2); the softmax path and final outputs are fp32.
  - Small weight tensors are replicated; the final FC stays node-local.
"""

import numpy as np

N_NODES = 50000
N_EDGES = 1000000
NC = 8
NPC = N_NODES // NC          # 6250 owned nodes per core
NT = 49                      # node tiles per core (128 nodes each)
NPCP = NT * 128              # 6272 padded nodes per core
NTOT = NPCP * NC             # 50176 table rows
PADROW = 6250                # core 0's first pad slot (el = -1e30)
NEG_SLOPE = 0.2

COLS_MAX = 128               # max G*Kc gathered per chunk
G_MAX = 1                    # max tiles per chunk (1 = zero slot padding)

# (Fin, H, D) per GAT layer
LAYERS = [(25, 4, 10), (40, 4, 25), (100, 1, 50)]

_cache = {}


def _patch_tile_drain():
    """walrus in this toolchain rejects instructions carrying more than one
    semaphore wait; split the TileContext tail drain's waits onto
    single-wait NOPs."""
    import concourse.tile as tile_mod
    import concourse.mybir as mybir
    from concourse.vector_clock import ScopedClock

    if getattr(tile_mod.TileContext, "_drain_patched", False):
        return

    def _patched(self, tick_clock, wait_clock):
        nc = self.nc
        drain_inst = nc.sync.drain()
        wait_clock.add_sem_waits(
            drain_inst.ins, ScopedClock({None: tick_clock.global_clock})
        )
        si = drain_inst.ins.sync_info
        waits = list(si.on_wait or []) if si is not None else []
        if len(waits) > 1:
            si.on_wait.clear()
            bb = nc.cur_bb.bb
            assert bb.instructions[-1] is drain_inst.ins
            bb.instructions.pop()
            for w in waits:
                nop = nc.sync.nop(nofuse=True, hint="drain_wait_split")
                if nop.ins.sync_info is None:
                    nop.ins.sync_info = mybir.SyncInfo(on_wait=[w], on_update=[])
                else:
                    nop.ins.sync_info.on_wait.append(w)
            bb.add_instruction(drain_inst.ins)
        nc.all_engine_barrier()
        assert self.sems is not None
        popped = nc._tile_sem_poison_stack.pop()
        assert popped is self._sem_poison
        nc.clear_and_free_semaphores(list(self.sems.allocated().values()))
        nc.all_engine_barrier()

    tile_mod.TileContext._drain_and_barrier = _patched
    tile_mod.TileContext._drain_patched = True


def _preprocess(src, dst):
    """Node->core assignment, chunked degree layout, gather indices."""
    deg = np.bincount(dst, minlength=N_NODES)
    rank = np.argsort(-deg, kind="stable")
    node_core = np.empty(N_NODES, np.int64)
    node_loc = np.empty(N_NODES, np.int64)
    node_core[rank] = np.arange(N_NODES) % NC
    node_loc[rank] = np.arange(N_NODES) // NC

    loc_deg = np.zeros((NC, NPCP), np.int64)
    loc_deg[node_core, node_loc] = deg
    K_t = [int(loc_deg[:, t * 128:(t + 1) * 128].max()) for t in range(NT)]

    # group tiles into chunks with a shared Kc (K_t is non-increasing)
    chunks = []                      # (t0, G, Kc, col0)
    t = 0
    col = 0
    while t < NT:
        Kc = max(1, K_t[t])
        G = 1
        while (t + G < NT and G < G_MAX and (G + 1) * Kc <= COLS_MAX):
            G += 1
        chunks.append((t, G, Kc, col))
        col += G * Kc
        t += G
    C_tot = col

    # table layout: two contiguous core-major blocks split at the chunk
    # boundary SPLIT_T, so each block is one contiguous AllGather output
    split_t = max(t0 + G for (t0, G, _, _) in chunks if t0 + G <= 36)
    r1 = split_t * 128
    glob = np.where(node_loc < r1,
                    node_core * r1 + node_loc,
                    NC * r1 + node_core * (NPCP - r1) + (node_loc - r1))
    padrow = NC * r1 + (NPC - r1)        # core 0's first pad slot

    idx_arr = np.full((NC, 128, C_tot), padrow, np.int32)
    key = node_core[dst] * NPCP + node_loc[dst]
    eorder = np.argsort(key, kind="stable")
    ksorted = key[eorder]
    starts = np.searchsorted(ksorted, np.arange(NC * NPCP))
    ends = np.searchsorted(ksorted, np.arange(NC * NPCP) + 1)
    gsorted = glob[src[eorder]]
    for c in range(NC):
        base = c * NPCP
        for (t0, G, Kc, col0) in chunks:
            for g in range(G):
                tt = t0 + g
                cb = col0 + g * Kc
                for u in range(128):
                    loc = base + tt * 128 + u
                    s0, s1 = starts[loc], ends[loc]
                    if s1 > s0:
                        idx_arr[c, u, cb:cb + (s1 - s0)] = gsorted[s0:s1]
    return node_core, node_loc, tuple(chunks), C_tot, idx_arr


def _proj_matrix(W, al, ar):
    """P = [W; L^T W; R^T W] so that x @ P.T = [h | el | er] per node."""
    H, D = al.shape
    HD = H * D
    L = np.zeros((HD, H), np.float32)
    R = np.zeros((HD, H), np.float32)
    for h in range(H):
        L[h * D:(h + 1) * D, h] = al[h]
        R[h * D:(h + 1) * D, h] = ar[h]
    return np.vstack([W, L.T @ W, R.T @ W]).astype(np.float32)


def _build(chunks, C_tot):
    import concourse.bass as bass
    import concourse.bacc as bacc
    import concourse.mybir as mybir
    from concourse.tile import TileContext
    from concourse.masks import make_identity

    _patch_tile_drain()

    nc = bacc.Bacc("TRN2", target_bir_lowering=False, debug=False, num_devices=NC)
    f32 = mybir.dt.float32
    bf16 = mybir.dt.bfloat16

    # ---- external inputs ----
    xin = nc.dram_tensor("xin", [25, NPCP], bf16, kind="ExternalInput")
    idx = nc.dram_tensor("idx", [128, C_tot], mybir.dt.int32, kind="ExternalInput")
    pTs = []
    bts = []
    for l, (Fin, H, D) in enumerate(LAYERS):
        pTs.append(nc.dram_tensor(f"pT{l}", [Fin, H * D + 2 * H], bf16,
                                  kind="ExternalInput"))
        bts.append(nc.dram_tensor(f"bias{l}", [128, H * D], f32,
                                  kind="ExternalInput"))
    fcT = nc.dram_tensor("fcT", [50, 93], bf16, kind="ExternalInput")
    fcb = nc.dram_tensor("fcb", [128, 93], f32, kind="ExternalInput")
    padm = nc.dram_tensor("padm", [128, 4], f32, kind="ExternalInput")
    out93 = nc.dram_tensor("out93", [NPCP, 93], f32, kind="ExternalOutput")

    # ---- internal DRAM: halo pieces and gathered tables (bf16) ----
    pieces = []
    tables = []
    DRs = []
    for l, (Fin, H, D) in enumerate(LAYERS):
        DR = H * D + H
        DR += DR & 1                     # even row width (4B-aligned rows)
        DRs.append(DR)
        pieces.append(nc.dram_tensor(f"piece{l}", [NPCP, DR], bf16,
                                     kind="Internal"))
        tables.append(nc.dram_tensor(f"table{l}", [NTOT, DR], bf16,
                                     kind="Internal", addr_space="Shared"))

    with TileContext(nc) as tc:
        with (
            tc.tile_pool(name="const", bufs=1) as cpool,
            tc.tile_pool(name="pc", bufs=3) as pcp,
            tc.tile_pool(name="gt", bufs=4) as gtp,
            tc.tile_pool(name="mm", bufs=3) as mmp,
            tc.tile_pool(name="wk", bufs=3) as wkp,
            tc.tile_pool(name="psa", bufs=4, space="PSUM") as psa,
            tc.tile_pool(name="psb", bufs=4, space="PSUM") as psb,
        ):
            ident = cpool.tile([128, 128], f32, tag="ident")
            make_identity(nc, ident[:])
            it = cpool.tile([128, C_tot], mybir.dt.int32, tag="idx")
            nc.sync.dma_start(it[:], idx[:])
            pt_t = []
            b_t = []
            for l, (Fin, H, D) in enumerate(LAYERS):
                HD = H * D
                p = cpool.tile([Fin, HD + 2 * H], bf16, tag=f"pt{l}")
                nc.sync.dma_start(p[:], pTs[l][:])
                pt_t.append(p)
                b = cpool.tile([128, HD], f32, tag=f"b{l}")
                nc.sync.dma_start(b[:], bts[l][:])
                b_t.append(b)
            fct = cpool.tile([50, 93], bf16, tag="fct")
            nc.sync.dma_start(fct[:], fcT[:])
            fcbt = cpool.tile([128, 93], f32, tag="fcbt")
            nc.sync.dma_start(fcbt[:], fcb[:])
            padt = cpool.tile([128, 4], f32, tag="padt")
            nc.sync.dma_start(padt[:], padm[:])
            ers = [cpool.tile([128, NT, LAYERS[l][1]], bf16, tag=f"er{l}",
                              name=f"er{l}") for l in range(3)]
            xt0 = cpool.tile([25, NPCP], bf16, tag="xt0")
            nc.sync.dma_start(xt0[:], xin[:])
            xts = [xt0,
                   cpool.tile([40, NPCP], bf16, tag="xt1", name="xt1"),
                   cpool.tile([100, NPCP], bf16, tag="xt2", name="xt2"),
                   cpool.tile([50, NPCP], bf16, tag="xt3", name="xt3")]

            def emit_proj(l, tiles):
                """Project own nodes of `tiles` for layer l: [h|el] piece rows
                plus the er table kept in SBUF."""
                Fin, H, D = LAYERS[l]
                HD = H * D
                PR = HD + 2 * H
                DR = DRs[l]
                for s in tiles:
                    cp = psa.tile([128, PR], f32, tag="ps_a", space="PSUM")
                    nc.tensor.matmul(cp[:],
                                     lhsT=xts[l][:, s * 128:(s + 1) * 128],
                                     rhs=pt_t[l][:], start=True, stop=True)
                    pe = pcp.tile([128, DR], bf16, tag="pe")
                    if s == NT - 1:
                        # pad-node slots get el = -1e30 so their alpha is 0
                        nc.scalar.copy(pe[:, 0:HD], cp[:, 0:HD])
                        nc.vector.tensor_tensor(
                            out=pe[:, HD:HD + H], in0=cp[:, HD:HD + H],
                            in1=padt[:, 0:H], op=mybir.AluOpType.add)
                    else:
                        nc.scalar.copy(pe[:, 0:HD + H], cp[:, 0:HD + H])
                    if DR > HD + H:
                        nc.vector.memset(pe[:, HD + H:DR], 0.0)
                    nc.scalar.copy(ers[l][:, s, :], cp[:, HD + H:PR])
                    nc.sync.dma_start(pieces[l][s * 128:(s + 1) * 128, :],
                                      pe[:])

            def emit_fc(tiles):
                for s in tiles:
                    fp = psa.tile([128, 93], f32, tag="ps_a", space="PSUM")
                    nc.tensor.matmul(fp[:],
                                     lhsT=xts[3][:, s * 128:(s + 1) * 128],
                                     rhs=fct[:], start=True, stop=True)
                    fo = pcp.tile([128, 93], f32, tag="fo")
                    nc.vector.tensor_tensor(out=fo[:], in0=fp[:], in1=fcbt[:],
                                            op=mybir.AluOpType.add)
                    nc.sync.dma_start(out93[s * 128:(s + 1) * 128, :], fo[:])

            # split point for the pipelined halo exchange: the latest chunk
            # boundary at or before tile 36 (~3/4 of the piece rows go out
            # early, the small remainder right after the last projection)
            SPLIT_T = max(t0 + G for (t0, G, _, _) in chunks if t0 + G <= 36)
            R1 = SPLIT_T * 128

            def emit_ag(l, part):
                """AllGather one contiguous block of layer l's halo table."""
                if part == 0:
                    ins_ = pieces[l][0:R1, :]
                    outs_ = tables[l][0:NC * R1, :]
                else:
                    ins_ = pieces[l][R1:NPCP, :]
                    outs_ = tables[l][NC * R1:NTOT, :]
                nc.gpsimd.collective_compute(
                    "AllGather", mybir.AluOpType.bypass,
                    replica_groups=[list(range(NC))],
                    ins=[ins_], outs=[outs_],
                )

            # startup: the first table block's exchange overlaps the tail
            # of the initial projection
            emit_proj(0, range(SPLIT_T))
            emit_ag(0, 0)
            emit_proj(0, range(SPLIT_T, NT))
            emit_ag(0, 1)

            for l, (Fin, H, D) in enumerate(LAYERS):
                HD = H * D
                DR = DRs[l]
                xt_next = xts[l + 1]
                table = tables[l]
                er_sb = ers[l]

                # ---- edge phase, one chunk at a time; the next layer's
                # projection (or the FC) is emitted per chunk so it runs
                # under the remaining gathers ----
                for (t0, G, Kc, col0) in chunks:
                    GK = G * Kc
                    gt = gtp.tile([128, GK, DR], bf16, tag="gt")
                    for j in range(GK):
                        nc.gpsimd.indirect_dma_start(
                            out=gt[:, j, :], out_offset=None, in_=table[:],
                            in_offset=bass.IndirectOffsetOnAxis(
                                ap=it[:, col0 + j:col0 + j + 1], axis=0),
                        )
                    # e = prelu(el_src + er_dst); alpha = exp(e)
                    e1 = wkp.tile([128, GK, H], f32, tag="e1")
                    nc.vector.tensor_tensor(
                        out=e1[:].rearrange("p (g k) h -> p g k h", g=G),
                        in0=gt[:, :, HD:HD + H].rearrange(
                            "p (g k) h -> p g k h", g=G),
                        in1=er_sb[:, t0:t0 + G, None, :].broadcast_to(
                            [128, G, Kc, H]),
                        op=mybir.AluOpType.add)
                    e2 = wkp.tile([128, GK, H], f32, tag="e2")
                    nc.scalar.activation(
                        e2[:].rearrange("p q h -> p (q h)"),
                        e1[:].rearrange("p q h -> p (q h)"),
                        mybir.ActivationFunctionType.Prelu, alpha=NEG_SLOPE)
                    alpha = wkp.tile([128, GK, H], bf16, tag="al")
                    nc.scalar.activation(
                        alpha[:].rearrange("p q h -> p (q h)"),
                        e2[:].rearrange("p q h -> p (q h)"),
                        mybir.ActivationFunctionType.Exp)
                    # denominators per (tile, head)
                    den = wkp.tile([128, G, H], f32, tag="den")
                    nc.vector.tensor_reduce(
                        out=den[:],
                        in_=alpha[:].rearrange("p (g k) h -> p g h k", g=G),
                        axis=mybir.AxisListType.X, op=mybir.AluOpType.add)
                    nc.vector.tensor_scalar_max(
                        den[:].rearrange("p g h -> p (g h)"),
                        den[:].rearrange("p g h -> p (g h)"), 1e-30)
                    rden = wkp.tile([128, G, H], f32, tag="rden")
                    nc.vector.reciprocal(
                        rden[:].rearrange("p g h -> p (g h)"),
                        den[:].rearrange("p g h -> p (g h)"))
                    # weighted messages, tree-reduced over edge slots
                    m = mmp.tile([128, GK, HD], bf16, tag="m")
                    nc.vector.tensor_tensor(
                        out=m[:].rearrange("p q (h d) -> p q h d", h=H),
                        in0=gt[:, :, 0:HD].rearrange(
                            "p q (h d) -> p q h d", h=H),
                        in1=alpha[:, :, :, None].broadcast_to([128, GK, H, D]),
                        op=mybir.AluOpType.mult)
                    m4 = m[:].rearrange("p (g k) f -> p g k f", g=G)
                    w = Kc
                    while w > 1:
                        h2 = w // 2
                        nc.vector.tensor_tensor(
                            out=m4[:, :, 0:h2, :], in0=m4[:, :, 0:h2, :],
                            in1=m4[:, :, h2:2 * h2, :],
                            op=mybir.AluOpType.add)
                        if w & 1:
                            nc.vector.tensor_tensor(
                                out=m4[:, :, 0, :], in0=m4[:, :, 0, :],
                                in1=m4[:, :, w - 1, :],
                                op=mybir.AluOpType.add)
                        w = h2
                    # normalize, bias, relu
                    o = wkp.tile([128, G, HD], f32, tag="o")
                    nc.vector.tensor_tensor(
                        out=o[:].rearrange("p g (h d) -> p g h d", h=H),
                        in0=m4[:, :, 0, :].rearrange(
                            "p g (h d) -> p g h d", h=H),
                        in1=rden[:, :, :, None].broadcast_to([128, G, H, D]),
                        op=mybir.AluOpType.mult)
                    nc.vector.tensor_tensor(
                        out=o[:], in0=o[:],
                        in1=b_t[l][:, None, :].broadcast_to([128, G, HD]),
                        op=mybir.AluOpType.add)
                    nc.vector.tensor_scalar_max(
                        o[:].rearrange("p g f -> p (g f)"),
                        o[:].rearrange("p g f -> p (g f)"), 0.0)
                    # transpose per tile into next layer's lhsT, then
                    # immediately project (or FC) those tiles
                    for g in range(G):
                        tp = psb.tile([HD, 128], f32, tag="ps_b", space="PSUM")
                        nc.tensor.transpose(tp[:], o[:, g, :], ident[:])
                        nc.scalar.copy(
                            xt_next[:, (t0 + g) * 128:(t0 + g + 1) * 128],
                            tp[:])
                    if l < 2:
                        emit_proj(l + 1, range(t0, t0 + G))
                        # first half of the next halo exchange launches as
                        # soon as its piece rows exist, hiding its transfer
                        # under this layer's remaining gathers
                        if t0 + G == SPLIT_T:
                            emit_ag(l + 1, 0)
                    else:
                        emit_fc(range(t0, t0 + G))

                if l < 2:
                    emit_ag(l + 1, 1)

    nc.compile()
    return nc


def _prepare(inputs):
    import ml_dtypes
    bf16 = ml_dtypes.bfloat16

    src = np.ascontiguousarray(np.asarray(inputs["src"], dtype=np.int32))
    dst = np.ascontiguousarray(np.asarray(inputs["dst"], dtype=np.int32))
    feats = np.asarray(inputs["features"], dtype=np.float32)

    node_core, node_loc, chunks, C_tot, idx_arr = _preprocess(src, dst)

    ck = (chunks, C_tot)
    if ck not in _cache:
        _cache[ck] = _build(chunks, C_tot)
    nc = _cache[ck]

    pTl = []
    btl = []
    for l in range(3):
        W = np.asarray(inputs[f"W{l + 1}"], np.float32)
        al = np.asarray(inputs[f"al{l + 1}"], np.float32)
        ar = np.asarray(inputs[f"ar{l + 1}"], np.float32)
        b = np.asarray(inputs[f"b{l + 1}"], np.float32)
        P = _proj_matrix(W, al, ar)                          # [PR, Fin]
        pTl.append(np.ascontiguousarray(P.T.astype(bf16)))   # [Fin, PR]
        btl.append(np.ascontiguousarray(
            np.tile(b[None, :], (128, 1)).astype(np.float32)))
    fcw = np.asarray(inputs["fc_w"], np.float32)             # [93, 50]
    fcb = np.asarray(inputs["fc_b"], np.float32)
    fcT = np.ascontiguousarray(fcw.T.astype(bf16))           # [50, 93]
    fcb128 = np.ascontiguousarray(
        np.tile(fcb[None, :], (128, 1)).astype(np.float32))
    # last tile: partitions >= NPC - 48*128 (=106) are pad-node slots
    padmask = np.zeros((128, 4), np.float32)
    padmask[NPC - 48 * 128:, :] = -1e30

    in_maps = []
    for c in range(NC):
        xfm = np.zeros((25, NPCP), np.float32)
        sel = node_core == c
        xfm[:, node_loc[sel]] = feats[sel].T
        m = {"xin": xfm.astype(bf16), "idx": np.ascontiguousarray(idx_arr[c]),
             "fcT": fcT, "fcb": fcb128, "padm": padmask}
        for l in range(3):
            m[f"pT{l}"] = pTl[l]
            m[f"bias{l}"] = btl[l]
        in_maps.append(m)
    return nc, in_maps, node_core, node_loc


def _unshard(results, node_core, node_loc):
    out = np.zeros((N_NODES, 93), np.float32)
    for c in range(NC):
        o = np.asarray(results[c]["out93"], np.float32)      # [NPCP, 93]
        sel = node_core == c
        out[np.where(sel)[0]] = o[node_loc[sel]]
    return out


def kernel(**inputs):
    from concourse import bass_utils

    nc, in_maps, node_core, node_loc = _prepare(inputs)
    res = bass_utils.run_bass_kernel_spmd(nc, in_maps, core_ids=list(range(NC)))
    return _unshard(res.results, node_core, node_loc)


# revision 31
# speedup vs baseline: 1.1195x; 1.0059x over previous
"""Distributed 3-layer GAT + FC kernel for Trainium2 (8 NeuronCores).

Strategy (graph/data parallel, per the sharding hint):
  - Nodes are assigned to the 8 cores by in-degree rank interleaving
    (rank r -> core r%8, local slot r//8), so every core gets an almost
    identical degree profile and the SPMD program is shared.
  - Each core owns its nodes' incoming edges, laid out degree-bucketed:
    dst node = partition, incoming-edge slot = free-dim column.  One
    indirect DMA (128 rows, one per partition) gathers the source-node
    table rows for one edge-slot column.  Tiles (128 dst nodes) are
    grouped into chunks with a uniform slot count Kc so the edge math
    runs as a few wide DVE/ACT ops per chunk.
  - Per layer, each core projects its own nodes ([h | el | er] in one
    fused matmul; the el/er attention reductions are folded into the
    projection matrix host-side), then an AllGather replicates the
    bf16 [h | el] node table to every core (the halo exchange).
    The projection of layer l+1 (and the final FC) is emitted INSIDE
    layer l's edge loop, right after each chunk produces its activation
    tiles, so PE/ACT/DMA run it concurrently with the remaining
    gathers and the AllGather launches right after the last gather.
  - Edge softmax: the segment max is skipped (|e| <= ~10 on this data,
    exp cannot overflow) and the alpha normalization is applied after
    the weighted tree-reduction:
        out[n] = (sum_e exp(e_e) h[src_e]) / (sum_e exp(e_e)).
    Padding edge slots point at a pad-node table row whose el is set
    to -1e30, so exp() contributes exactly 0.
  - Table/gather/message datapath is bf16 (relative-error budget
    2e-2); the softmax path and final outputs are fp32.
  - Small weight tensors are replicated; the final FC stays node-local.
"""

import numpy as np

N_NODES = 50000
N_EDGES = 1000000
NC = 8
NPC = N_NODES // NC          # 6250 owned nodes per core
NT = 49                      # node tiles per core (128 nodes each)
NPCP = NT * 128              # 6272 padded nodes per core
NTOT = NPCP * NC             # 50176 table rows
PADROW = 6250                # core 0's first pad slot (el = -1e30)
NEG_SLOPE = 0.2

COLS_MAX = 128               # max G*Kc gathered per chunk
G_MAX = 1                    # max tiles per chunk (1 = zero slot padding)

# (Fin, H, D) per GAT layer
LAYERS = [(25, 4, 10), (40, 4, 25), (100, 1, 50)]

_cache = {}


def _patch_tile_drain():
    """walrus in this toolchain rejects instructions carrying more than one
    semaphore wait; split the TileContext tail drain's waits onto
    single-wait NOPs."""
    import concourse.tile as tile_mod
    import concourse.mybir as mybir
    from concourse.vector_clock import ScopedClock

    if getattr(tile_mod.TileContext, "_drain_patched", False):
        return

    def _patched(self, tick_clock, wait_clock):
        nc = self.nc
        drain_inst = nc.sync.drain()
        wait_clock.add_sem_waits(
            drain_inst.ins, ScopedClock({None: tick_clock.global_clock})
        )
        si = drain_inst.ins.sync_info
        waits = list(si.on_wait or []) if si is not None else []
        if len(waits) > 1:
            si.on_wait.clear()
            bb = nc.cur_bb.bb
            assert bb.instructions[-1] is drain_inst.ins
            bb.instructions.pop()
            for w in waits:
                nop = nc.sync.nop(nofuse=True, hint="drain_wait_split")
                if nop.ins.sync_info is None:
                    nop.ins.sync_info = mybir.SyncInfo(on_wait=[w], on_update=[])
                else:
                    nop.ins.sync_info.on_wait.append(w)
            bb.add_instruction(drain_inst.ins)
        nc.all_engine_barrier()
        assert self.sems is not None
        popped = nc._tile_sem_poison_stack.pop()
        assert popped is self._sem_poison
        nc.clear_and_free_semaphores(list(self.sems.allocated().values()))
        nc.all_engine_barrier()

    tile_mod.TileContext._drain_and_barrier = _patched
    tile_mod.TileContext._drain_patched = True


def _preprocess(src, dst):
    """Node->core assignment, chunked degree layout, gather indices."""
    deg = np.bincount(dst, minlength=N_NODES)
    rank = np.argsort(-deg, kind="stable")
    node_core = np.empty(N_NODES, np.int64)
    node_loc = np.empty(N_NODES, np.int64)
    node_core[rank] = np.arange(N_NODES) % NC
    node_loc[rank] = np.arange(N_NODES) // NC

    loc_deg = np.zeros((NC, NPCP), np.int64)
    loc_deg[node_core, node_loc] = deg
    K_t = [int(loc_deg[:, t * 128:(t + 1) * 128].max()) for t in range(NT)]

    # group tiles into chunks with a shared Kc (K_t is non-increasing)
    chunks = []                      # (t0, G, Kc, col0)
    t = 0
    col = 0
    while t < NT:
        Kc = max(1, K_t[t])
        G = 1
        while (t + G < NT and G < G_MAX and (G + 1) * Kc <= COLS_MAX):
            G += 1
        chunks.append((t, G, Kc, col))
        col += G * Kc
        t += G
    C_tot = col

    # table layout: two contiguous core-major blocks split at the chunk
    # boundary SPLIT_T, so each block is one contiguous AllGather output
    split_t = max(t0 + G for (t0, G, _, _) in chunks if t0 + G <= 36)
    r1 = split_t * 128
    glob = np.where(node_loc < r1,
                    node_core * r1 + node_loc,
                    NC * r1 + node_core * (NPCP - r1) + (node_loc - r1))
    padrow = NC * r1 + (NPC - r1)        # core 0's first pad slot

    idx_arr = np.full((NC, 128, C_tot), padrow, np.int32)
    key = node_core[dst] * NPCP + node_loc[dst]
    eorder = np.argsort(key, kind="stable")
    ksorted = key[eorder]
    starts = np.searchsorted(ksorted, np.arange(NC * NPCP))
    ends = np.searchsorted(ksorted, np.arange(NC * NPCP) + 1)
    gsorted = glob[src[eorder]]
    for c in range(NC):
        base = c * NPCP
        for (t0, G, Kc, col0) in chunks:
            for g in range(G):
                tt = t0 + g
                cb = col0 + g * Kc
                for u in range(128):
                    loc = base + tt * 128 + u
                    s0, s1 = starts[loc], ends[loc]
                    if s1 > s0:
                        idx_arr[c, u, cb:cb + (s1 - s0)] = gsorted[s0:s1]
    return node_core, node_loc, tuple(chunks), C_tot, idx_arr


def _proj_matrix(W, al, ar):
    """P = [W; L^T W; R^T W] so that x @ P.T = [h | el | er] per node."""
    H, D = al.shape
    HD = H * D
    L = np.zeros((HD, H), np.float32)
    R = np.zeros((HD, H), np.float32)
    for h in range(H):
        L[h * D:(h + 1) * D, h] = al[h]
        R[h * D:(h + 1) * D, h] = ar[h]
    return np.vstack([W, L.T @ W, R.T @ W]).astype(np.float32)


def _build(chunks, C_tot):
    import concourse.bass as bass
    import concourse.bacc as bacc
    import concourse.mybir as mybir
    from concourse.tile import TileContext
    from concourse.masks import make_identity

    _patch_tile_drain()

    nc = bacc.Bacc("TRN2", target_bir_lowering=False, debug=False, num_devices=NC)
    f32 = mybir.dt.float32
    bf16 = mybir.dt.bfloat16

    # ---- external inputs ----
    xin = nc.dram_tensor("xin", [25, NPCP], bf16, kind="ExternalInput")
    idx = nc.dram_tensor("idx", [128, C_tot], mybir.dt.int32, kind="ExternalInput")
    pTs = []
    bts = []
    for l, (Fin, H, D) in enumerate(LAYERS):
        pTs.append(nc.dram_tensor(f"pT{l}", [Fin, H * D + 2 * H], bf16,
                                  kind="ExternalInput"))
        bts.append(nc.dram_tensor(f"bias{l}", [128, H * D], f32,
                                  kind="ExternalInput"))
    fcT = nc.dram_tensor("fcT", [50, 93], bf16, kind="ExternalInput")
    fcb = nc.dram_tensor("fcb", [128, 93], f32, kind="ExternalInput")
    padm = nc.dram_tensor("padm", [128, 4], f32, kind="ExternalInput")
    out93 = nc.dram_tensor("out93", [NPCP, 93], f32, kind="ExternalOutput")

    # ---- internal DRAM: halo pieces and gathered tables (bf16) ----
    pieces = []
    tables = []
    DRs = []
    for l, (Fin, H, D) in enumerate(LAYERS):
        DR = H * D + H
        DR += DR & 1                     # even row width (4B-aligned rows)
        DRs.append(DR)
        pieces.append(nc.dram_tensor(f"piece{l}", [NPCP, DR], bf16,
                                     kind="Internal"))
        tables.append(nc.dram_tensor(f"table{l}", [NTOT, DR], bf16,
                                     kind="Internal", addr_space="Shared"))

    with TileContext(nc) as tc:
        with (
            tc.tile_pool(name="const", bufs=1) as cpool,
            tc.tile_pool(name="pc", bufs=3) as pcp,
            tc.tile_pool(name="gt", bufs=6) as gtp,
            tc.tile_pool(name="mm", bufs=4) as mmp,
            tc.tile_pool(name="wk", bufs=3) as wkp,
            tc.tile_pool(name="psa", bufs=4, space="PSUM") as psa,
            tc.tile_pool(name="psb", bufs=4, space="PSUM") as psb,
        ):
            ident = cpool.tile([128, 128], f32, tag="ident")
            make_identity(nc, ident[:])
            it = cpool.tile([128, C_tot], mybir.dt.int32, tag="idx")
            nc.sync.dma_start(it[:], idx[:])
            pt_t = []
            b_t = []
            for l, (Fin, H, D) in enumerate(LAYERS):
                HD = H * D
                p = cpool.tile([Fin, HD + 2 * H], bf16, tag=f"pt{l}")
                nc.sync.dma_start(p[:], pTs[l][:])
                pt_t.append(p)
                b = cpool.tile([128, HD], f32, tag=f"b{l}")
                nc.sync.dma_start(b[:], bts[l][:])
                b_t.append(b)
            fct = cpool.tile([50, 93], bf16, tag="fct")
            nc.sync.dma_start(fct[:], fcT[:])
            fcbt = cpool.tile([128, 93], f32, tag="fcbt")
            nc.sync.dma_start(fcbt[:], fcb[:])
            padt = cpool.tile([128, 4], f32, tag="padt")
            nc.sync.dma_start(padt[:], padm[:])
            ers = [cpool.tile([128, NT, LAYERS[l][1]], bf16, tag=f"er{l}",
                              name=f"er{l}") for l in range(3)]
            xt0 = cpool.tile([25, NPCP], bf16, tag="xt0")
            nc.sync.dma_start(xt0[:], xin[:])
            xts = [xt0,
                   cpool.tile([40, NPCP], bf16, tag="xt1", name="xt1"),
                   cpool.tile([100, NPCP], bf16, tag="xt2", name="xt2"),
                   cpool.tile([50, NPCP], bf16, tag="xt3", name="xt3")]

            def emit_proj(l, tiles):
                """Project own nodes of `tiles` for layer l: [h|el] piece rows
                plus the er table kept in SBUF."""
                Fin, H, D = LAYERS[l]
                HD = H * D
                PR = HD + 2 * H
                DR = DRs[l]
                for s in tiles:
                    cp = psa.tile([128, PR], f32, tag="ps_a", space="PSUM")
                    nc.tensor.matmul(cp[:],
                                     lhsT=xts[l][:, s * 128:(s + 1) * 128],
                                     rhs=pt_t[l][:], start=True, stop=True)
                    pe = pcp.tile([128, DR], bf16, tag="pe")
                    if s == NT - 1:
                        # pad-node slots get el = -1e30 so their alpha is 0
                        nc.vector.tensor_copy(pe[:, 0:HD], cp[:, 0:HD])
                        nc.vector.tensor_tensor(
                            out=pe[:, HD:HD + H], in0=cp[:, HD:HD + H],
                            in1=padt[:, 0:H], op=mybir.AluOpType.add)
                    else:
                        nc.vector.tensor_copy(pe[:, 0:HD + H],
                                              cp[:, 0:HD + H])
                    if DR > HD + H:
                        nc.vector.memset(pe[:, HD + H:DR], 0.0)
                    nc.vector.tensor_copy(ers[l][:, s, :], cp[:, HD + H:PR])
                    nc.sync.dma_start(pieces[l][s * 128:(s + 1) * 128, :],
                                      pe[:])

            def emit_fc(tiles):
                for s in tiles:
                    fp = psa.tile([128, 93], f32, tag="ps_a", space="PSUM")
                    nc.tensor.matmul(fp[:],
                                     lhsT=xts[3][:, s * 128:(s + 1) * 128],
                                     rhs=fct[:], start=True, stop=True)
                    fo = pcp.tile([128, 93], f32, tag="fo")
                    nc.vector.tensor_tensor(out=fo[:], in0=fp[:], in1=fcbt[:],
                                            op=mybir.AluOpType.add)
                    nc.sync.dma_start(out93[s * 128:(s + 1) * 128, :], fo[:])

            # split point for the pipelined halo exchange: the latest chunk
            # boundary at or before tile 36 (~3/4 of the piece rows go out
            # early, the small remainder right after the last projection)
            SPLIT_T = max(t0 + G for (t0, G, _, _) in chunks if t0 + G <= 36)
            R1 = SPLIT_T * 128

            def emit_ag(l, part):
                """AllGather one contiguous block of layer l's halo table."""
                if part == 0:
                    ins_ = pieces[l][0:R1, :]
                    outs_ = tables[l][0:NC * R1, :]
                else:
                    ins_ = pieces[l][R1:NPCP, :]
                    outs_ = tables[l][NC * R1:NTOT, :]
                nc.gpsimd.collective_compute(
                    "AllGather", mybir.AluOpType.bypass,
                    replica_groups=[list(range(NC))],
                    ins=[ins_], outs=[outs_],
                )

            # startup: the first table block's exchange overlaps the tail
            # of the initial projection
            emit_proj(0, range(SPLIT_T))
            emit_ag(0, 0)
            emit_proj(0, range(SPLIT_T, NT))
            emit_ag(0, 1)

            for l, (Fin, H, D) in enumerate(LAYERS):
                HD = H * D
                DR = DRs[l]
                xt_next = xts[l + 1]
                table = tables[l]
                er_sb = ers[l]

                # ---- edge phase, one chunk at a time; the next layer's
                # projection (or the FC) is emitted per chunk so it runs
                # under the remaining gathers ----
                for (t0, G, Kc, col0) in chunks:
                    GK = G * Kc
                    gt = gtp.tile([128, GK, DR], bf16, tag="gt")
                    for j in range(GK):
                        nc.gpsimd.indirect_dma_start(
                            out=gt[:, j, :], out_offset=None, in_=table[:],
                            in_offset=bass.IndirectOffsetOnAxis(
                                ap=it[:, col0 + j:col0 + j + 1], axis=0),
                        )
                    # e = prelu(el_src + er_dst); alpha = exp(e)
                    e1 = wkp.tile([128, GK, H], f32, tag="e1")
                    nc.vector.tensor_tensor(
                        out=e1[:].rearrange("p (g k) h -> p g k h", g=G),
                        in0=gt[:, :, HD:HD + H].rearrange(
                            "p (g k) h -> p g k h", g=G),
                        in1=er_sb[:, t0:t0 + G, None, :].broadcast_to(
                            [128, G, Kc, H]),
                        op=mybir.AluOpType.add)
                    e2 = wkp.tile([128, GK, H], f32, tag="e2")
                    nc.scalar.activation(
                        e2[:].rearrange("p q h -> p (q h)"),
                        e1[:].rearrange("p q h -> p (q h)"),
                        mybir.ActivationFunctionType.Prelu, alpha=NEG_SLOPE)
                    alpha = wkp.tile([128, GK, H], bf16, tag="al")
                    nc.scalar.activation(
                        alpha[:].rearrange("p q h -> p (q h)"),
                        e2[:].rearrange("p q h -> p (q h)"),
                        mybir.ActivationFunctionType.Exp)
                    # denominators per (tile, head)
                    den = wkp.tile([128, G, H], f32, tag="den")
                    nc.vector.tensor_reduce(
                        out=den[:],
                        in_=alpha[:].rearrange("p (g k) h -> p g h k", g=G),
                        axis=mybir.AxisListType.X, op=mybir.AluOpType.add)
                    nc.vector.tensor_scalar_max(
                        den[:].rearrange("p g h -> p (g h)"),
                        den[:].rearrange("p g h -> p (g h)"), 1e-30)
                    rden = wkp.tile([128, G, H], f32, tag="rden")
                    nc.vector.reciprocal(
                        rden[:].rearrange("p g h -> p (g h)"),
                        den[:].rearrange("p g h -> p (g h)"))
                    # weighted messages, tree-reduced over edge slots
                    m = mmp.tile([128, GK, HD], bf16, tag="m")
                    nc.vector.tensor_tensor(
                        out=m[:].rearrange("p q (h d) -> p q h d", h=H),
                        in0=gt[:, :, 0:HD].rearrange(
                            "p q (h d) -> p q h d", h=H),
                        in1=alpha[:, :, :, None].broadcast_to([128, GK, H, D]),
                        op=mybir.AluOpType.mult)
                    m4 = m[:].rearrange("p (g k) f -> p g k f", g=G)
                    w = Kc
                    while w > 1:
                        h2 = w // 2
                        nc.vector.tensor_tensor(
                            out=m4[:, :, 0:h2, :], in0=m4[:, :, 0:h2, :],
                            in1=m4[:, :, h2:2 * h2, :],
                            op=mybir.AluOpType.add)
                        if w & 1:
                            nc.vector.tensor_tensor(
                                out=m4[:, :, 0, :], in0=m4[:, :, 0, :],
                                in1=m4[:, :, w - 1, :],
                                op=mybir.AluOpType.add)
                        w = h2
                    # normalize, bias, relu
                    o = wkp.tile([128, G, HD], f32, tag="o")
                    nc.vector.tensor_tensor(
                        out=o[:].rearrange("p g (h d) -> p g h d", h=H),
                        in0=m4[:, :, 0, :].rearrange(
                            "p g (h d) -> p g h d", h=H),
                        in1=rden[:, :, :, None].broadcast_to([128, G, H, D]),
                        op=mybir.AluOpType.mult)
                    nc.vector.tensor_tensor(
                        out=o[:], in0=o[:],
                        in1=b_t[l][:, None, :].broadcast_to([128, G, HD]),
                        op=mybir.AluOpType.add)
                    nc.vector.tensor_scalar_max(
                        o[:].rearrange("p g f -> p (g f)"),
                        o[:].rearrange("p g f -> p (g f)"), 0.0)
                    # transpose per tile into next layer's lhsT, then
                    # immediately project (or FC) those tiles
                    for g in range(G):
                        tp = psb.tile([HD, 128], f32, tag="ps_b", space="PSUM")
                        nc.tensor.transpose(tp[:], o[:, g, :], ident[:])
                        nc.scalar.copy(
                            xt_next[:, (t0 + g) * 128:(t0 + g + 1) * 128],
                            tp[:])
                    if l < 2:
                        emit_proj(l + 1, range(t0, t0 + G))
                        # first half of the next halo exchange launches as
                        # soon as its piece rows exist, hiding its transfer
                        # under this layer's remaining gathers
                        if t0 + G == SPLIT_T:
                            emit_ag(l + 1, 0)
                    else:
                        emit_fc(range(t0, t0 + G))

                if l < 2:
                    emit_ag(l + 1, 1)

    nc.compile()
    return nc


def _prepare(inputs):
    import ml_dtypes
    bf16 = ml_dtypes.bfloat16

    src = np.ascontiguousarray(np.asarray(inputs["src"], dtype=np.int32))
    dst = np.ascontiguousarray(np.asarray(inputs["dst"], dtype=np.int32))
    feats = np.asarray(inputs["features"], dtype=np.float32)

    node_core, node_loc, chunks, C_tot, idx_arr = _preprocess(src, dst)

    ck = (chunks, C_tot)
    if ck not in _cache:
        _cache[ck] = _build(chunks, C_tot)
    nc = _cache[ck]

    pTl = []
    btl = []
    for l in range(3):
        W = np.asarray(inputs[f"W{l + 1}"], np.float32)
        al = np.asarray(inputs[f"al{l + 1}"], np.float32)
        ar = np.asarray(inputs[f"ar{l + 1}"], np.float32)
        b = np.asarray(inputs[f"b{l + 1}"], np.float32)
        P = _proj_matrix(W, al, ar)                          # [PR, Fin]
        pTl.append(np.ascontiguousarray(P.T.astype(bf16)))   # [Fin, PR]
        btl.append(np.ascontiguousarray(
            np.tile(b[None, :], (128, 1)).astype(np.float32)))
    fcw = np.asarray(inputs["fc_w"], np.float32)             # [93, 50]
    fcb = np.asarray(inputs["fc_b"], np.float32)
    fcT = np.ascontiguousarray(fcw.T.astype(bf16))           # [50, 93]
    fcb128 = np.ascontiguousarray(
        np.tile(fcb[None, :], (128, 1)).astype(np.float32))
    # last tile: partitions >= NPC - 48*128 (=106) are pad-node slots
    padmask = np.zeros((128, 4), np.float32)
    padmask[NPC - 48 * 128:, :] = -1e30

    in_maps = []
    for c in range(NC):
        xfm = np.zeros((25, NPCP), np.float32)
        sel = node_core == c
        xfm[:, node_loc[sel]] = feats[sel].T
        m = {"xin": xfm.astype(bf16), "idx": np.ascontiguousarray(idx_arr[c]),
             "fcT": fcT, "fcb": fcb128, "padm": padmask}
        for l in range(3):
            m[f"pT{l}"] = pTl[l]
            m[f"bias{l}"] = btl[l]
        in_maps.append(m)
    return nc, in_maps, node_core, node_loc


def _unshard(results, node_core, node_loc):
    out = np.zeros((N_NODES, 93), np.float32)
    for c in range(NC):
        o = np.asarray(results[c]["out93"], np.float32)      # [NPCP, 93]
        sel = node_core == c
        out[np.where(sel)[0]] = o[node_loc[sel]]
    return out


def kernel(**inputs):
    from concourse import bass_utils

    nc, in_maps, node_core, node_loc = _prepare(inputs)
    res = bass_utils.run_bass_kernel_spmd(nc, in_maps, core_ids=list(range(NC)))
    return _unshard(res.results, node_core, node_loc)
